# revision 1
# baseline (speedup 1.0000x reference)
"""Trainium2 Bass kernel for nn_Block1_87144886436577 (vq_codebook).

The reference's giant jacobians (jacrev through a 2-layer strided conv
net) collapse analytically: each output pixel o sees a 10x10 input
patch, so the per-o gradient image e_total[b,o] is a 10x10 patch
    e_patch[:, (b,o)] = sum_k2 W1s_tap[k2].T @ (mask1_tap * (W2_tap @ r2))
computed with 16 tap matmuls. The argmin over o per input pixel reduces
to a 9-candidate compare (3x3 covering windows) plus the structural-zero
tie rule of the reference (first non-covering o when the min is 0). The
scatter-accumulated y_masked similarly collapses to tap matmuls over a
selection-masked x. Irregular gathers use GPSIMD indirect_copy with
host-precomputed constant index streams; everything else is plain
<=3-dim DMAs and strided SBUF views.

Single-core program; all 8 cores run identical replicas (B=2, ~us-scale
work: cross-core collectives would cost more than they save). Output is
read from core 0.
"""
import sys

import numpy as np

for _p in ("/opt/trn_rl_repo",):
    if _p not in sys.path:
        sys.path.insert(0, _p)

import concourse.bass as bass
import concourse.mybir as mybir
import concourse.tile as tile

F32 = mybir.dt.float32
U16 = mybir.dt.uint16
AF = mybir.ActivationFunctionType
ALU = mybir.AluOpType
AX = mybir.AxisListType
AP = bass.AP

N_CORES = 8


def v(t, off, pat):
    """Custom-view AP over a tile (t = AP returned by pool.tile)."""
    return AP(t.tensor, t.offset + off, pat)


def _e(r):
    return 1 if r >= 1 else 0


def _consts():
    """Host-precomputed constant tensors (input-independent)."""
    ident128 = np.eye(128, dtype=np.float32)
    oidx128 = np.tile((np.arange(128) % 64).astype(np.float32)[None, :],
                      (128, 1))

    # xsel gather index streams: 8 tiles, per tile idx table [128,16] u16.
    # tile t: k2y=t//2, k2x_half=t%2; partition p=k2x'*64+k1y*16+k1x*4+ci;
    # group g=p//16=k2x'*4+k1y. stream j in [0,256): j<128 -> x value at
    # free f=j=(b,oy,ox); j>=128 -> sel value (+2888). k1x lives in the
    # data-row shift, ci in the data-row content.
    idxX = np.zeros((8, 128, 8), np.uint16)
    for t in range(8):
        k2y, k2xh = t // 2, t % 2
        for g in range(8):
            k2xp = g // 4
            k1y = g % 4
            k2x = 2 * k2xh + k2xp
            for j in range(128):
                b, oy, ox = j // 64, (j % 64) // 8, j % 8
                idxX[t, 16 * g + j % 16, j // 16] = (
                    b * 1444 + (4 * oy + 2 * k2y + k1y) * 38
                    + 4 * ox + 2 * k2x)

    # sel gather: 2 tiles, rows r=(k2y%2)*64+k2x*16+k1y*4+k1x (s=k2y//2);
    # group g=(k2y%2)*4+k2x; (k1y,k1x) live in the data-row shift.
    idxS = np.zeros((2, 128, 8), np.uint16)
    for s in range(2):
        for g in range(8):
            k2yp, k2x = g // 4, g % 4
            k2y = s * 2 + k2yp
            for j in range(128):
                b, oy, ox = j // 64, (j % 64) // 8, j % 8
                idxS[s, 16 * g + j % 16, j // 16] = (
                    b * 1444 + (4 * oy + 2 * k2y) * 38 + 4 * ox + 2 * k2x)

    # expansion matrices: xsel-tile-row p <- selm2 tile s row r
    emat = np.zeros((8, 128, 128), np.float32)
    for t in range(8):
        k2y, k2xh = t // 2, t % 2
        for p in range(128):
            k2xp, k1y, k1x = p // 64, (p % 64) // 16, p % 4
            k2x = 2 * k2xh + k2xp
            r = (k2y % 2) * 64 + k2x * 16 + k1y * 4 + k1x
            emat[t, r, p] = 1.0

    # E9 gather: partition p = r*32+h*16+b*8+q (iy=4q+r, ix=16h+ixl),
    # group g=r*2+h. data row = ed4p[b, q+e(r) : +3 oyp-rows] flat 3600
    # ([3 oyp, 12 oxp, 100 dydx]). stream j in [0,144): f=(ixl,jj).
    idxE = np.zeros((128, 9), np.uint16)
    for r in range(4):
        for h in range(2):
            g = r * 2 + h
            for j in range(144):
                ixl, jj = j // 9, j % 9
                jy, jx = jj // 3, jj % 3
                t_ = ixl % 4
                s = 4 * h + ixl // 4
                dy = r - 4 * _e(r) + 4 * jy + 3
                dx = t_ - 4 * _e(t_) + 4 * jx + 3
                oxp = s + _e(t_) - jx + 2
                if 0 <= dy < 10 and 0 <= dx < 10:
                    idx = (2 - jy) * 1200 + oxp * 100 + dy * 10 + dx
                else:
                    idx = 0  # guaranteed-zero pad cell
                idxE[16 * g + j % 16, j // 16] = idx

    # candidate o-index (3000 = invalid) and first-noncovering-o tables
    oidx9 = np.full((128, 144), 3000.0, np.float32)
    zc = np.zeros((128, 16), np.float32)
    for r in range(4):
        for h in range(2):
            for b in range(2):
                for q in range(8):
                    p = r * 32 + h * 16 + b * 8 + q
                    iy = 4 * q + r
                    for ixl in range(16):
                        ix = 16 * h + ixl
                        t_ = ix % 4
                        s = ix // 4
                        for jj in range(9):
                            jy, jx = jj // 3, jj % 3
                            oy = q + _e(r) - jy
                            ox = s + _e(t_) - jx
                            dy = iy - 4 * oy + 3
                            dx = ix - 4 * ox + 3
                            if (0 <= oy < 8 and 0 <= ox < 8
                                    and 0 <= dy < 10 and 0 <= dx < 10):
                                oidx9[p, ixl * 9 + jj] = oy * 8 + ox
                        # first o whose patch does NOT cover this pixel
                        for o in range(64):
                            oy, ox = o // 8, o % 8
                            if not (0 <= iy - 4 * oy + 3 < 10
                                    and 0 <= ix - 4 * ox + 3 < 10):
                                zc[p, ixl] = float(o)
                                break
    return {"ident128": ident128, "oidx128": oidx128,
            "idxX": idxX, "idxS": idxS, "emat": emat,
            "idxE": idxE, "oidx9": oidx9, "zc": zc}


def build_program(nc):
    x_d = nc.declare_dram_parameter("x", [2, 3, 32, 32], F32, isOutput=False)
    w1_d = nc.declare_dram_parameter("w1", [32, 3, 4, 4], F32, isOutput=False)
    b1_d = nc.declare_dram_parameter("b1", [32], F32, isOutput=False)
    w2_d = nc.declare_dram_parameter("w2", [64, 32, 4, 4], F32, isOutput=False)
    b2_d = nc.declare_dram_parameter("b2", [64], F32, isOutput=False)
    k_d = nc.declare_dram_parameter("K", [512, 64], F32, isOutput=False)
    v_d = nc.declare_dram_parameter("V", [512, 64], F32, isOutput=False)
    id_d = nc.declare_dram_parameter("ident128", [128, 128], F32,
                                     isOutput=False)
    oi_d = nc.declare_dram_parameter("oidx128", [128, 128], F32,
                                     isOutput=False)
    ixx_d = nc.declare_dram_parameter("idxX", [8, 128, 8], U16,
                                      isOutput=False)
    ixs_d = nc.declare_dram_parameter("idxS", [2, 128, 8], U16,
                                      isOutput=False)
    em_d = nc.declare_dram_parameter("emat", [8, 128, 128], F32,
                                     isOutput=False)
    ixe_d = nc.declare_dram_parameter("idxE", [128, 9], U16, isOutput=False)
    oi9_d = nc.declare_dram_parameter("oidx9", [128, 144], F32,
                                      isOutput=False)
    zc_d = nc.declare_dram_parameter("zc", [128, 16], F32, isOutput=False)
    out_d = nc.declare_dram_parameter("out", [2, 64, 8, 8], F32,
                                      isOutput=True)

    with tile.TileContext(nc) as tc:
        with (
            tc.tile_pool(name="const", bufs=1) as cpool,
            tc.tile_pool(name="work", bufs=1) as wpool,
            tc.tile_pool(name="psA", bufs=2, space="PSUM") as psA,
            tc.tile_pool(name="psB", bufs=2, space="PSUM") as psB,
            tc.tile_pool(name="psC", bufs=1, space="PSUM") as psC,
            tc.tile_pool(name="dram", bufs=1, space="DRAM") as dpool,
        ):
            dma = nc.sync.dma_start

            # ---- DRAM scratch (flat; element offsets computed by hand) ----
            x_pad = dpool.tile([11584], F32)   # [2b,4ci,38,38] + tail pad
            sel_pad = dpool.tile([3040], F32)  # [2b,38,38] + shift tail
            ed4p = dpool.tile([2, 12, 12, 100], F32)

            # ---- zero / constant tiles ----
            z128 = cpool.tile([128, 2400], F32)
            nc.gpsimd.memset(z128[:], 0.0)
            zneg = cpool.tile([2, 1520], F32)
            nc.gpsimd.memset(zneg[:], -1.0)

            ixX = cpool.tile([128, 64], U16)
            dma(v(ixX, 0, [[ixX.ap[0][0], 128], [8, 8], [1, 8]]),
                AP(ixx_d, 0, [[8, 128], [1024, 8], [1, 8]]))
            ident = cpool.tile([128, 128], F32)
            dma(ident[:], id_d[:])

            # early zero-fills / pads for late DRAM scratch
            dma(v(ed4p, 0, [[2400, 12], [1, 2400]]), z128[0:12, :])
            dma(v(sel_pad, 0, [[1520, 2], [1, 1520]]), zneg[:])
            # ---- weight staging ----
            w1sb = wpool.tile([32, 48], F32)          # [m, (ci,k1)]
            dma(w1sb[:], AP(w1_d, 0, [[48, 32], [1, 48]]))
            w1taps = wpool.tile([3, 512], F32)        # [ci, (m,k1)]
            dma(w1taps[:], AP(w1_d, 0, [[16, 3], [48, 32], [1, 16]]))
            w2sb = wpool.tile([32, 1024], F32)        # [m, (c,k2)]
            dma(w2sb[:], AP(w2_d, 0, [[16, 32], [512, 64], [1, 16]]))
            w2c2 = wpool.tile([64, 512], F32)         # [c, (m,k2)]
            dma(w2c2[:], AP(w2_d, 0, [[512, 64], [16, 32], [1, 16]]))
            b1t = wpool.tile([32, 1], F32)
            dma(b1t[:], AP(b1_d, 0, [[1, 32], [1, 1]]))
            b2t = wpool.tile([64, 1], F32)
            dma(b2t[:], AP(b2_d, 0, [[1, 64], [1, 1]]))

            # w1fp [128, 64->(k1y,ci4,k1x) rows x2, m]: via on-chip build
            w1sb2 = wpool.tile([32, 48], F32)   # [m, (ci,k1)]
            dma(w1sb2[:], AP(w1_d, 0, [[48, 32], [16, 3], [1, 16]]))
            w1sb2p = wpool.tile([32, 64], F32)  # [m, (k1y,ci4,k1x)]
            nc.vector.memset(w1sb2p[:], 0.0)
            nc.vector.tensor_copy(
                v(w1sb2p, 0, [[w1sb2p.ap[0][0], 32], [16, 4], [4, 3], [1, 4]]),
                v(w1sb2, 0, [[w1sb2.ap[0][0], 32], [4, 4], [16, 3], [1, 4]]))
            w1fp_ps = psB.tile([64, 32], F32, tag="psB", name="w1fp_ps")
            nc.tensor.transpose(w1fp_ps[:], w1sb2p[:], ident[0:32, 0:32])
            w1fp = wpool.tile([128, 32], F32)
            nc.scalar.copy(w1fp[0:64, :], w1fp_ps[:])
            dma(w1fp[64:128, :], w1fp[0:64, :])

            # K^T and V in SBUF
            kt_sb = wpool.tile([64, 512], F32)
            v_sb = wpool.tile([128, 256], F32)
            for t in range(4):
                k_tile = wpool.tile([128, 64], F32, tag="k_tile",
                                    name="k_tile")
                dma(k_tile[:], AP(k_d, t * 8192, [[64, 128], [1, 64]]))
                kt_ps = psB.tile([64, 128], F32, tag="psB", name="kt_ps")
                nc.tensor.transpose(kt_ps[:], k_tile[:], ident[:])
                nc.scalar.copy(kt_sb[:, t * 128:(t + 1) * 128], kt_ps[:])
                dma(v_sb[:, t * 64:(t + 1) * 64],
                    AP(v_d, t * 8192, [[64, 128], [1, 64]]))


            # ---- padded x image in DRAM + SBUF ----
            dma(v(x_pad, 0, [[1448, 8], [1, 1448]]), z128[0:8, 0:1448])
            for ci in range(3):
                dma(v(x_pad, ci * 1444 + 117, [[5776, 2], [38, 32], [1, 32]]),
                    AP(x_d, ci * 1024, [[3072, 2], [1, 1024]]))
            xp2 = wpool.tile([3, 2888], F32)  # [ci, (b, 38x38)]
            dma(xp2[:], v(x_pad, 0, [[1444, 3], [5776, 2], [1, 1444]]))
            xpitch = xp2.ap[0][0]

            # x-side gather data (independent of sel -> overlaps phases A/B)
            data_x = wpool.tile([128, 2888], F32)
            for ci in range(4):
                dma(data_x[ci * 4:ci * 4 + 4, :],
                    v(x_pad, ci * 1444, [[1, 4], [5776, 2], [1, 1444]]))
            for d in (16, 32, 64):
                dma(data_x[d:2 * d, :], data_x[0:d, :])
            xg3 = []
            for t in range(8):
                xg = wpool.tile([128, 128], F32, name=f"xg{t}")
                nc.gpsimd.indirect_copy(
                    v(xg, 0, [[xg.ap[0][0], 128], [1, 128], [1, 1]]),
                    data_x[:], ixX[:, t * 8:(t + 1) * 8], True)
                xg3.append(xg)


            # ---- Phase A: forward ----
            y1ps = psA.tile([32, 512], F32, tag="psA", name="y1ps")
            for k in range(16):
                k1y, k1x = k // 4, k % 4
                nc.tensor.matmul(
                    y1ps[:],
                    v(w1taps, 4 * k1y + k1x,
                      [[w1taps.ap[0][0], 3], [16, 32]]),
                    v(xp2, 78 + 38 * k1y + k1x,
                      [[xpitch, 3], [1444, 2], [76, 16], [2, 16]]),
                    start=(k == 0), stop=(k == 15))
            y1sb = wpool.tile([32, 512], F32)  # [m, (b,py,px)]
            nc.scalar.activation(y1sb[:], y1ps[:], AF.Relu, bias=b1t[:])

            y1p = wpool.tile([32, 648], F32)   # [m, (b,18,18)] padded
            nc.vector.memset(y1p[:], 0.0)
            ypitch = y1p.ap[0][0]
            nc.vector.tensor_copy(
                v(y1p, 19, [[ypitch, 32], [324, 2], [18, 16], [1, 16]]),
                v(y1sb, 0,
                  [[y1sb.ap[0][0], 32], [256, 2], [16, 16], [1, 16]]))
            m1p = wpool.tile([32, 648], F32)
            nc.vector.tensor_scalar(m1p[:], y1p[:], 0.0, None, ALU.is_gt)

            def tapview(tl, k2, pitch):
                k2y, k2x = k2 // 4, k2 % 4
                return v(tl, 18 * k2y + k2x,
                         [[pitch, 32], [324, 2], [36, 8], [2, 8]])

            ypre = psA.tile([64, 128], F32, tag="psA", name="ypre")
            for k2 in range(16):
                nc.tensor.matmul(
                    ypre[:],
                    v(w2sb, k2, [[w2sb.ap[0][0], 32], [16, 64]]),
                    tapview(y1p, k2, ypitch),
                    start=(k2 == 0), stop=(k2 == 15))
            yT = wpool.tile([64, 128], F32)    # [c, (b,o)]
            nc.scalar.activation(yT[:], ypre[:], AF.Relu, bias=b2t[:])
            m2T = wpool.tile([64, 128], F32)
            nc.vector.tensor_scalar(m2T[:], yT[:], 0.0, None, ALU.is_gt)

            # deferred const loads (needed only mid/late)
            oidx = cpool.tile([128, 128], F32)
            dma(oidx[:], oi_d[:])
            ixS = cpool.tile([128, 16], U16)
            dma(v(ixS, 0, [[ixS.ap[0][0], 128], [8, 2], [1, 8]]),
                AP(ixs_d, 0, [[8, 128], [1024, 2], [1, 8]]))
            emt = cpool.tile([128, 1024], F32)
            dma(v(emt, 0, [[emt.ap[0][0], 128], [128, 8], [1, 128]]),
                AP(em_d, 0, [[128, 128], [16384, 8], [1, 128]]))
            ixE = cpool.tile([128, 9], U16)
            dma(ixE[:], ixe_d[:])
            oidx9 = cpool.tile([128, 144], F32)
            dma(oidx9[:], oi9_d[:])
            zct = cpool.tile([128, 16], F32)
            dma(zct[:], zc_d[:])


            def hopfield(src, tag):
                """src [c 64, (b,o) 128] SBUF -> yq [(b,o) 128, c 64] SBUF."""
                a_ps = psA.tile([128, 512], F32, tag="psA",
                                name=f"a_ps{tag}")
                nc.tensor.matmul(a_ps[:], src, kt_sb[:], start=True,
                                 stop=True)
                rmax = wpool.tile([128, 1], F32, name=f"rmax{tag}")
                nc.vector.tensor_reduce(rmax[:], a_ps[:], AX.X, ALU.max)
                negbm = wpool.tile([128, 1], F32, name=f"negbm{tag}")
                nc.vector.tensor_scalar(negbm[:], rmax[:], -0.125, None,
                                        ALU.mult)
                p_sb = wpool.tile([128, 512], F32, name=f"p_sb{tag}")
                ssum = wpool.tile([128, 1], F32, name=f"ssum{tag}")
                nc.scalar.activation(p_sb[:], a_ps[:], AF.Exp, bias=negbm[:],
                                     scale=0.125, accum_out=ssum[:])
                rec = wpool.tile([128, 1], F32, name=f"rec{tag}")
                nc.vector.reciprocal(rec[:], ssum[:])
                nc.vector.tensor_scalar(p_sb[:], p_sb[:], rec[:], None,
                                        ALU.mult)
                yq_ps = psC.tile([128, 64], F32, tag="psC",
                                 name=f"yq_ps{tag}")
                for t in range(4):
                    pt_ps = psB.tile([128, 128], F32, tag="psB",
                                     name=f"pt_ps{tag}{t}")
                    nc.tensor.transpose(pt_ps[:],
                                        p_sb[:, t * 128:(t + 1) * 128],
                                        ident[:])
                    pt_sb = wpool.tile([128, 128], F32, tag="pt_sb",
                                       name=f"pt_sb{tag}{t}")
                    nc.scalar.copy(pt_sb[:], pt_ps[:])
                    nc.tensor.matmul(yq_ps[:], pt_sb[:],
                                     v_sb[:, t * 64:(t + 1) * 64],
                                     start=(t == 0), stop=(t == 3))
                yq_sb = wpool.tile([128, 64], F32, name=f"yq_sb{tag}")
                nc.scalar.copy(yq_sb[:], yq_ps[:])
                return yq_sb

            yq1 = hopfield(yT[:], "h1")

            yqT_ps = psB.tile([64, 128], F32, tag="psB", name="yqT_ps")
            nc.tensor.transpose(yqT_ps[:], yq1[:], ident[:])
            r2T = wpool.tile([64, 128], F32)
            nc.vector.scalar_tensor_tensor(r2T[:], yqT_ps[:], -1.0, yT[:],
                                           ALU.mult, ALU.add)
            nc.vector.tensor_mul(r2T[:], r2T[:], m2T[:])

            # ---- Phase B: e_patch + argmin ----
            w1s16 = wpool.tile([32, 16], F32)
            nc.vector.tensor_reduce(
                w1s16[:],
                v(w1sb, 0, [[w1sb.ap[0][0], 32], [1, 16], [16, 3]]),
                AX.X, ALU.add)
            w1si = wpool.tile([32, 256], F32)  # w1s 16x16 zero-pad image
            nc.vector.memset(w1si[:], 0.0)
            nc.vector.tensor_copy(
                v(w1si, 102, [[w1si.ap[0][0], 32], [16, 4], [1, 4]]),
                w1s16[:])
            # 16 contiguous 100-wide taps (walrus: stationary operand
            # must be a single free dim)
            w1stp = wpool.tile([32, 1600], F32)
            for k2y in range(4):
                nc.vector.tensor_copy(
                    v(w1stp, k2y * 400,
                      [[w1stp.ap[0][0], 32], [100, 4], [10, 10], [1, 10]]),
                    v(w1si, (6 - 2 * k2y) * 16 + 6,
                      [[w1si.ap[0][0], 32], [-2, 4], [16, 10], [1, 10]]))

            g1m = [wpool.tile([32, 128], F32, name=f"g1m{k2}")
                   for k2 in range(16)]
            for k2 in range(16):
                g1ps = psB.tile([32, 128], F32, tag="psB", name=f"g1ps{k2}")
                nc.tensor.matmul(
                    g1ps[:],
                    v(w2c2, k2, [[w2c2.ap[0][0], 64], [16, 32]]),
                    r2T[:], start=True, stop=True)
                nc.vector.tensor_tensor(g1m[k2][:], g1ps[:],
                                        tapview(m1p, k2, ypitch), ALU.mult)

            ep_ps = psA.tile([100, 128], F32, tag="psA", name="ep_ps")
            for k2 in range(16):
                k2y, k2x = k2 // 4, k2 % 4
                nc.tensor.matmul(
                    ep_ps[:],
                    w1stp[:, k2 * 100:(k2 + 1) * 100],
                    g1m[k2][:], start=(k2 == 0), stop=(k2 == 15))
            ep_sb = wpool.tile([100, 128], F32)
            nc.scalar.copy(ep_sb[:], ep_ps[:])
            ep2_ps = psB.tile([128, 100], F32, tag="psB", name="ep2_ps")
            nc.tensor.transpose(ep2_ps[:], ep_sb[:], ident[0:100, 0:100])
            ep2 = wpool.tile([128, 100], F32)  # [(b,oy,ox), (dy,dx)]
            nc.scalar.copy(ep2[:], ep2_ps[:])

            # scatter patches to DRAM (padded per-o layout), gather E9 rows
            for b in range(2):
                dma(v(ed4p, b * 14400 + 2 * 1200 + 2 * 100,
                      [[1200, 8], [100, 8], [1, 100]]),
                    ep2[b * 64:(b + 1) * 64, :])

            data_e = wpool.tile([128, 3600], F32)
            for r in range(4):
                for h in range(2):
                    dma(data_e[r * 32 + h * 16:r * 32 + h * 16 + 16, :],
                        v(ed4p, _e(r) * 1200,
                          [[14400, 2], [1200, 8], [1, 3600]]))
            e9 = wpool.tile([128, 144], F32)
            e9pitch = e9.ap[0][0]
            nc.gpsimd.indirect_copy(
                v(e9, 0, [[e9pitch, 128], [1, 144], [1, 1]]),
                data_e[:], ixE[:], True)

            # argmin with reference tie semantics
            mincand = wpool.tile([128, 16], F32)
            nc.vector.tensor_reduce(
                mincand[:], v(e9, 0, [[e9pitch, 128], [9, 16], [1, 9]]),
                AX.X, ALU.min)
            mstar = wpool.tile([128, 16], F32)
            nc.vector.tensor_scalar(mstar[:], mincand[:], 0.0, None, ALU.min)
            eq9 = wpool.tile([128, 144], F32)
            nc.vector.tensor_tensor(
                v(eq9, 0, [[eq9.ap[0][0], 128], [9, 16], [1, 9]]),
                v(e9, 0, [[e9pitch, 128], [9, 16], [1, 9]]),
                v(mstar, 0, [[mstar.ap[0][0], 128], [1, 16], [0, 9]]),
                ALU.is_equal)
            cs = wpool.tile([128, 144], F32)
            nc.vector.scalar_tensor_tensor(cs[:], eq9[:], -1000.0, oidx9[:],
                                           ALU.mult, ALU.add)
            minc2 = wpool.tile([128, 16], F32)
            nc.vector.tensor_reduce(
                minc2[:], v(cs, 0, [[cs.ap[0][0], 128], [9, 16], [1, 9]]),
                AX.X, ALU.min)
            zeq = wpool.tile([128, 16], F32)
            nc.vector.tensor_scalar(zeq[:], mstar[:], 0.0, None,
                                    ALU.is_equal)
            zsc = wpool.tile([128, 16], F32)
            nc.vector.scalar_tensor_tensor(zsc[:], zeq[:], -1000.0, zct[:],
                                           ALU.mult, ALU.add)
            sel16 = wpool.tile([128, 16], F32)
            nc.vector.tensor_tensor(sel16[:], minc2[:], zsc[:], ALU.min)
            nc.vector.tensor_scalar(sel16[:], sel16[:], 1000.0, None,
                                    ALU.add)

            # sel -> padded DRAM image (pad = -1, pre-filled)
            for r in range(4):
                for h in range(2):
                    dma(v(sel_pad, 117 + 38 * r + 16 * h,
                          [[1444, 2], [152, 8], [1, 16]]),
                        sel16[r * 32 + h * 16:r * 32 + h * 16 + 16, :])

            # ---- Phase C: sel gather + mask expansion ----
            # data rows: sel-img shifted by k1y*38+k1x (cycle of 16)
            data_s = wpool.tile([128, 2888], F32)
            for k1y in range(4):
                dma(data_s[k1y * 4:k1y * 4 + 4, :],
                    v(sel_pad, k1y * 38, [[1, 4], [1, 2888]]))
            for d in (16, 32, 64):
                dma(data_s[d:2 * d, :], data_s[0:d, :])
            selm2 = []
            for s in range(2):
                sg = wpool.tile([128, 128], F32, name=f"sg{s}")
                nc.gpsimd.indirect_copy(
                    v(sg, 0, [[sg.ap[0][0], 128], [1, 128], [1, 1]]),
                    data_s[:], ixS[:, s * 8:(s + 1) * 8], True)
                nc.vector.tensor_tensor(sg[:], sg[:], oidx[:], ALU.is_equal)
                selm2.append(sg)

            xsel = []
            for t in range(8):
                mx_ps = psB.tile([128, 128], F32, tag="psB", name=f"mx{t}")
                nc.tensor.matmul(mx_ps[:], emt[:, t * 128:(t + 1) * 128],
                                 selm2[(t // 2) // 2][:],
                                 start=True, stop=True)
                xs = wpool.tile([128, 128], F32, name=f"xs{t}")
                nc.vector.tensor_tensor(xs[:], xg3[t][:], mx_ps[:],
                                        ALU.mult)
                xsel.append(xs)

            zm = [wpool.tile([32, 128], F32, name=f"zm{k2}")
                  for k2 in range(16)]
            for k2 in range(16):
                k2y, k2x = k2 // 4, k2 % 4
                t = k2y * 2 + k2x // 2
                half = (k2x % 2) * 64
                z_ps = psB.tile([32, 128], F32, tag="psB", name=f"z_ps{k2}")
                nc.tensor.matmul(z_ps[:], w1fp[half:half + 64, :],
                                 xsel[t][half:half + 64, :],
                                 start=True, stop=True)
                nc.vector.tensor_tensor(zm[k2][:], z_ps[:],
                                        tapview(m1p, k2, ypitch), ALU.mult)

            ym_ps = psA.tile([128, 64], F32, tag="psA", name="ym_ps")
            for k2 in range(16):
                nc.tensor.matmul(
                    ym_ps[:], zm[k2][:],
                    v(w2sb, k2, [[w2sb.ap[0][0], 32], [16, 64]]),
                    start=(k2 == 0), stop=(k2 == 15))

            yTT_ps = psB.tile([128, 64], F32, tag="psB", name="yTT_ps")
            nc.tensor.transpose(yTT_ps[:], yT[:], ident[0:64, 0:64])
            m2g = wpool.tile([128, 64], F32)
            nc.vector.tensor_scalar(m2g[:], yTT_ps[:], 0.0, None, ALU.is_gt)
            ymm = wpool.tile([128, 64], F32)
            nc.vector.tensor_tensor(ymm[:], ym_ps[:], m2g[:], ALU.mult)

            t2_ps = psB.tile([64, 128], F32, tag="psB", name="t2_ps")
            nc.tensor.transpose(t2_ps[:], ymm[:], ident[:])
            ymmT = wpool.tile([64, 128], F32)
            nc.scalar.copy(ymmT[:], t2_ps[:])

            yq2 = hopfield(ymmT[:], "h2")

            tr_ps = psB.tile([64, 128], F32, tag="psB", name="tr_ps")
            nc.tensor.transpose(tr_ps[:], yq2[:], ident[:])
            outT = wpool.tile([64, 128], F32)
            nc.scalar.copy(outT[:], tr_ps[:])
            for b in range(2):
                dma(AP(out_d, b * 4096, [[64, 64], [8, 8], [1, 8]]),
                    outT[:, b * 64:(b + 1) * 64])

    return nc


_CACHE = {}


def kernel(**inputs) -> np.ndarray:
    from concourse.bass_utils import run_bass_kernel_spmd
    if "nc" not in _CACHE:
        from concourse import bacc
        nc = bacc.Bacc("TRN2", target_bir_lowering=False, debug=False,
                       num_devices=N_CORES)
        build_program(nc)
        nc.compile()
        _CACHE["nc"] = nc
        _CACHE["consts"] = _consts()
    nc = _CACHE["nc"]
    feed = {k: np.ascontiguousarray(np.asarray(val, np.float32))
            for k, val in inputs.items()}
    for k, val in _CACHE["consts"].items():
        feed[k] = val
    in_maps = [dict(feed) for _ in range(N_CORES)]
    res = run_bass_kernel_spmd(nc, in_maps, list(range(N_CORES)))
    return np.asarray(res.results[0]["out"], np.float32)



# revision 8
# speedup vs baseline: 1.8398x; 1.8398x over previous
"""Trainium2 Bass kernel for nn_Block1_87144886436577 (vq_codebook), v2.

Analytic collapse of the reference's jacobians (see v1 docstring), with:
- conv1 as ONE fp32 matmul (im2col rows (ci,k1y,k1x) built by 3 DMAs
  from the host-padded image, replicated 4x in partition blocks),
- conv2 as 4 fp32 matmuls contracting (k2x, m)=128 per k2y over a
  shifted-row copy of y1,
- phase-B backprop (g1/ep) packed into 8+4 fp32 matmuls via
  host-rearranged weights; fp32 is REQUIRED upstream of the argmin
  (sel margins are ~1e-3; bf16 flips selections),
- the e-patch/sel DRAM round-trips deduplicated + spread across the
  sync/scalar/gpsimd DMA queues, argmin on a 64-partition layout, a
  permutation matmul so sel scatters with a single 3-dim DMA,
- everything downstream of sel (mask expansion, masked conv taps,
  second hopfield) in bf16 with tap-pair packing,
- all weight repacks/transposes done host-side (pure layout + casts).

All 8 cores run identical replicas; output read from core 0.
"""
import sys

import numpy as np

for _p in ("/opt/trn_rl_repo",):
    if _p not in sys.path:
        sys.path.insert(0, _p)

import concourse.bass as bass
import concourse.mybir as mybir
import concourse.tile as tile

F32 = mybir.dt.float32
BF16 = mybir.dt.bfloat16
U16 = mybir.dt.uint16
AF = mybir.ActivationFunctionType
ALU = mybir.AluOpType
AX = mybir.AxisListType
AP = bass.AP

N_CORES = 8


def v(t, off, pat):
    """Custom-view AP over a tile (t = AP returned by pool.tile)."""
    return AP(t.tensor, t.offset + off, pat)


def _e(r):
    return 1 if r >= 1 else 0


def _tables():
    """Input-independent index/mask tables."""
    import ml_dtypes
    BF = ml_dtypes.bfloat16

    ident = np.eye(128, dtype=np.float32)
    identb = ident.astype(BF)
    oidx128 = np.tile((np.arange(128) % 64).astype(np.float32)[None, :],
                      (128, 1))

    # xsel gather streams (same as v1): tile t=(k2y,k2xh); partition
    # p=k2xp*64+k1y*16+k1x*4+ci; j<128 per tile: (b,oy,ox).
    idxX = np.zeros((8, 128, 8), np.uint16)
    for t in range(8):
        k2y, k2xh = t // 2, t % 2
        for g in range(8):
            k2xp = g // 4
            k1y = g % 4
            k2x = 2 * k2xh + k2xp
            for j in range(128):
                b, oy, ox = j // 64, (j % 64) // 8, j % 8
                idxX[t, 16 * g + j % 16, j // 16] = (
                    b * 1444 + (4 * oy + 2 * k2y + k1y) * 38
                    + 4 * ox + 2 * k2x)
    ixX = np.zeros((128, 64), np.uint16)
    for t in range(8):
        ixX[:, t * 8:(t + 1) * 8] = idxX[t]

    # sel gather streams (as v1)
    idxS = np.zeros((2, 128, 8), np.uint16)
    for s in range(2):
        for g in range(8):
            k2yp, k2x = g // 4, g % 4
            k2y = s * 2 + k2yp
            for j in range(128):
                b, oy, ox = j // 64, (j % 64) // 8, j % 8
                idxS[s, 16 * g + j % 16, j // 16] = (
                    b * 1444 + (4 * oy + 2 * k2y) * 38 + 4 * ox + 2 * k2x)
    ixS = np.zeros((128, 16), np.uint16)
    for s in range(2):
        ixS[:, s * 8:(s + 1) * 8] = idxS[s]

    # mask expansion matrices, bf16: emt8[r, t*128+p]
    emt8 = np.zeros((128, 1024), np.float32)
    for t in range(8):
        k2y, k2xh = t // 2, t % 2
        for p in range(128):
            k2xp, k1y, k1x = p // 64, (p % 64) // 16, p % 4
            k2x = 2 * k2xh + k2xp
            r = (k2y % 2) * 64 + k2x * 16 + k1y * 4 + k1x
            emt8[r, t * 128 + p] = 1.0
    emt8 = emt8.astype(BF)

    # E9 gather (v1 layout): p = r*32+h*16+b*8+q (iy=4q+r, ix=16h+ixl),
    # stream j = ixl*9 + jj. data row = ed4p[b, q+e(r) : +3 rows] flat.
    idxE = np.zeros((128, 9), np.uint16)
    oidx9 = np.full((128, 144), 3000.0, np.float32)
    zc128 = np.zeros((128, 16), np.float32)
    for r in range(4):
        for h in range(2):
            g = r * 2 + h
            for j in range(144):
                ixl, jj = j // 9, j % 9
                jy, jx = jj // 3, jj % 3
                t_ = ixl % 4
                s = 4 * h + ixl // 4
                dy = r - 4 * _e(r) + 4 * jy + 3
                dx = t_ - 4 * _e(t_) + 4 * jx + 3
                ox = s + _e(t_) - jx
                if 0 <= dy < 10 and 0 <= dx < 10:
                    idx = (2 - jy) * 1200 + (ox + 2) * 100 + dy * 10 + dx
                else:
                    idx = 0  # guaranteed-zero pad cell
                idxE[16 * g + j % 16, j // 16] = idx
    for r in range(4):
        for h in range(2):
            for b in range(2):
                for q in range(8):
                    p = r * 32 + h * 16 + b * 8 + q
                    iy = 4 * q + r
                    for ixl in range(16):
                        ix = 16 * h + ixl
                        t_ = ix % 4
                        s = ix // 4
                        for jj in range(9):
                            jy, jx = jj // 3, jj % 3
                            oy = q + _e(r) - jy
                            ox = s + _e(t_) - jx
                            dy = iy - 4 * oy + 3
                            dx = ix - 4 * ox + 3
                            if (0 <= oy < 8 and 0 <= ox < 8
                                    and 0 <= dy < 10 and 0 <= dx < 10):
                                oidx9[p, ixl * 9 + jj] = oy * 8 + ox
                        for o in range(64):
                            oy, ox = o // 8, o % 8
                            if not (0 <= iy - 4 * oy + 3 < 10
                                    and 0 <= ix - 4 * ox + 3 < 10):
                                zc128[p, ixl] = float(o)
                                break

    # permutations: sel16 rows (r,h,b,q) -> rows (b, iy), cols ix-halves
    permA = np.zeros((128, 64), np.float32)
    permB = np.zeros((128, 64), np.float32)
    for r in range(4):
        for h in range(2):
            for b in range(2):
                for q in range(8):
                    p = r * 32 + h * 16 + b * 8 + q
                    (permA if h == 0 else permB)[p, b * 32 + 4 * q + r] \
                        = 1.0

    neg1s = np.full((2, 1520), -1.0, np.float32)
    zed = np.zeros((12, 2400), np.float32)
    return {"ident": ident, "identb": identb, "oidx128": oidx128,
            "ixX": ixX, "ixS": ixS, "emt8": emt8, "idxE": idxE,
            "oidx9": oidx9, "zc128": zc128, "permA": permA,
            "permB": permB, "neg1s": neg1s, "zed": zed}


def prepare_feed(inputs):
    """Host-side layout/cast-only rearrangements of the inputs."""
    import ml_dtypes
    BF = ml_dtypes.bfloat16
    x = np.asarray(inputs["x"], np.float32)    # (2,3,32,32)
    w1 = np.asarray(inputs["w1"], np.float32)  # (32,3,4,4)
    b1 = np.asarray(inputs["b1"], np.float32)  # (32,)
    w2 = np.asarray(inputs["w2"], np.float32)  # (64,32,4,4)
    b2 = np.asarray(inputs["b2"], np.float32)  # (64,)
    K = np.asarray(inputs["K"], np.float32)    # (512,64)
    V = np.asarray(inputs["V"], np.float32)    # (512,64)

    f = dict(_TABLES)

    xpad = np.zeros((3, 3008), np.float32)
    img = np.zeros((3, 2, 38, 38), np.float32)
    img[:, :, 3:35, 3:35] = x.transpose(1, 0, 2, 3)
    xpad[:, 0:2888] = img.reshape(3, 2888)
    f["xpad3"] = xpad

    # data_x rows (ci,k1x): padded image shifted left by k1x; ci=3 zero.
    dx16 = np.zeros((16, 2888), np.float32)
    for ci in range(3):
        for k1x in range(4):
            dx16[ci * 4 + k1x, 0:2888 - k1x] = \
                f["xpad3"][ci, k1x:2888]
    f["dx16"] = dx16

    # conv1 weights: [ci*16+k1y*4+k1x, j*32+m], 4 dup col blocks
    w1t = w1.transpose(1, 2, 3, 0).reshape(48, 32)
    f["w1f4"] = np.tile(w1t, (1, 4)).copy()
    f["b1t4"] = np.tile(b1, 4).reshape(128, 1).copy()

    # conv2 weights: [k2x*32+m, k2y*64+c]
    W2p = np.zeros((128, 256), np.float32)
    for k2y in range(4):
        for k2x in range(4):
            W2p[k2x * 32:(k2x + 1) * 32, k2y * 64:(k2y + 1) * 64] = \
                w2[:, :, k2y, k2x].T
    f["W2p"] = W2p
    f["b2t"] = b2.reshape(64, 1).copy()

    f["ktK"] = K.T.copy()                       # [64, 512]
    f["vV"] = V.reshape(4, 128, 64).transpose(1, 0, 2).reshape(128, 256) \
        .copy()                                 # [128, (t,c)]
    f["ktb"] = f["ktK"].astype(BF)
    f["vb"] = f["vV"].astype(BF)

    # g1 pair weights: [c, k2*32+m]
    w2k2m = np.zeros((64, 512), np.float32)
    for k2 in range(16):
        k2y, k2x = k2 // 4, k2 % 4
        w2k2m[:, k2 * 32:(k2 + 1) * 32] = w2[:, :, k2y, k2x]
    f["w2k2m"] = w2k2m

    # w1 summed-tap images, built on chip from w1sb4
    f["w1sb4"] = np.tile(w1.reshape(32, 48), (4, 1)).copy()

    # z-pair blockdiag weights bf16 [128, 64]
    w1fp64 = np.zeros((64, 32), np.float32)
    for k1y in range(4):
        for ci in range(3):
            for k1x in range(4):
                w1fp64[k1y * 16 + ci * 4 + k1x, :] = w1[:, ci, k1y, k1x]
    bd = np.zeros((128, 64), np.float32)
    bd[0:64, 0:32] = w1fp64
    bd[64:128, 32:64] = w1fp64
    f["w1fpbd"] = bd.astype(BF)

    # ym pack weights bf16 [128, (g,c)]
    w2pk = np.zeros((128, 256), np.float32)
    for g in range(4):
        for k2x in range(4):
            w2pk[k2x * 32:(k2x + 1) * 32, g * 64:(g + 1) * 64] = \
                w2[:, :, g, k2x].T
    f["w2pk"] = w2pk.astype(BF)
    return f


_TABLES = _tables()


def build_program(nc):
    def P(name, shape, dt):
        return nc.declare_dram_parameter(name, shape, dt, isOutput=False)

    xpad_d = P("xpad3", [3, 3008], F32)
    dx16_d = P("dx16", [16, 2888], F32)
    w1f4_d = P("w1f4", [48, 128], F32)
    b1t4_d = P("b1t4", [128, 1], F32)
    W2p_d = P("W2p", [128, 256], F32)
    b2t_d = P("b2t", [64, 1], F32)
    ktK_d = P("ktK", [64, 512], F32)
    vV_d = P("vV", [128, 256], F32)
    ktb_d = P("ktb", [64, 512], BF16)
    vb_d = P("vb", [128, 256], BF16)
    w2k2m_d = P("w2k2m", [64, 512], F32)
    w1sb4_d = P("w1sb4", [128, 48], F32)
    w1fpbd_d = P("w1fpbd", [128, 64], BF16)
    w2pk_d = P("w2pk", [128, 256], BF16)
    ident_d = P("ident", [128, 128], F32)
    identb_d = P("identb", [128, 128], BF16)
    oidx128_d = P("oidx128", [128, 128], F32)
    ixX_d = P("ixX", [128, 64], U16)
    ixS_d = P("ixS", [128, 16], U16)
    emt8_d = P("emt8", [128, 1024], BF16)
    idxE_d = P("idxE", [128, 9], U16)
    oidx9_d = P("oidx9", [128, 144], F32)
    zc128_d = P("zc128", [128, 16], F32)
    permA_d = P("permA", [128, 64], F32)
    permB_d = P("permB", [128, 64], F32)
    neg1s_d = P("neg1s", [2, 1520], F32)
    zed_d = P("zed", [12, 2400], F32)
    out_d = nc.declare_dram_parameter("out", [2, 64, 8, 8], F32,
                                      isOutput=True)

    with tile.TileContext(nc) as tc:
        with (
            tc.tile_pool(name="const", bufs=1) as cpool,
            tc.tile_pool(name="work", bufs=1) as wpool,
            tc.tile_pool(name="psA", bufs=2, space="PSUM") as psA,
            tc.tile_pool(name="psB", bufs=4, space="PSUM") as psB,
            tc.tile_pool(name="psC", bufs=2, space="PSUM") as psC,
            tc.tile_pool(name="dram", bufs=1, space="DRAM") as dpool,
        ):
            dmaS = nc.sync.dma_start
            dmaA = nc.scalar.dma_start
            dmaG = nc.gpsimd.dma_start

            # ---- DRAM scratch ----
            ed4p = dpool.tile([2, 12, 12, 100], F32)
            sel_pad = dpool.tile([3040], F32)

            # ---- critical-path loads first (sync queue) ----
            w1f4 = wpool.tile([48, 128], F32)
            dmaS(w1f4[:], w1f4_d[:])
            X48 = wpool.tile([48, 2888], F32)
            for ci, dq in ((0, dmaS), (1, dmaA), (2, dmaG)):
                dq(X48[ci * 16:(ci + 1) * 16, :],
                   AP(xpad_d, ci * 3008, [[38, 4], [1, 4], [1, 2888]]))
            b1t4 = wpool.tile([128, 1], F32)
            dmaS(b1t4[:], b1t4_d[:])
            W2p = wpool.tile([128, 256], F32)
            dmaS(W2p[:], W2p_d[:])
            b2t = wpool.tile([64, 1], F32)
            dmaS(b2t[:], b2t_d[:])
            ktK = wpool.tile([64, 512], F32)
            dmaS(ktK[:], ktK_d[:])
            ident = cpool.tile([128, 128], F32)
            dmaS(ident[:], ident_d[:])
            vV = wpool.tile([128, 256], F32)
            dmaS(vV[:], vV_d[:])
            w2k2m = wpool.tile([64, 512], F32)
            dmaA(w2k2m[:], w2k2m_d[:])
            w1sb4 = wpool.tile([128, 48], F32)
            dmaA(w1sb4[:], w1sb4_d[:])

            # ---- non-critical loads (scalar/gpsimd queues) ----
            dmaA(v(ed4p, 0, [[2400, 12], [1, 2400]]), zed_d[:])
            dmaG(v(sel_pad, 0, [[1520, 2], [1, 1520]]), neg1s_d[:])
            ixX = cpool.tile([128, 64], U16)
            dmaG(ixX[:], ixX_d[:])
            dx = wpool.tile([128, 2888], F32)
            dmaG(dx[0:16, :], dx16_d[:])
            for d in (16, 32, 64):
                dmaG(dx[d:2 * d, :], dx[0:d, :])
            identb = cpool.tile([128, 128], BF16)
            dmaG(identb[:], identb_d[:])
            oidx128 = cpool.tile([128, 128], F32)
            dmaG(oidx128[:], oidx128_d[:])
            ixS = cpool.tile([128, 16], U16)
            dmaG(ixS[:], ixS_d[:])
            emt8 = cpool.tile([128, 1024], BF16)
            dmaG(emt8[:], emt8_d[:])
            idxE = cpool.tile([128, 9], U16)
            dmaA(idxE[:], idxE_d[:])
            oidx9 = cpool.tile([128, 144], F32)
            dmaA(oidx9[:], oidx9_d[:])
            zc128 = cpool.tile([128, 16], F32)
            dmaA(zc128[:], zc128_d[:])
            permA = cpool.tile([128, 64], F32)
            dmaA(permA[:], permA_d[:])
            permB = cpool.tile([128, 64], F32)
            dmaA(permB[:], permB_d[:])
            ktb = wpool.tile([64, 512], BF16)
            dmaA(ktb[:], ktb_d[:])
            vb = wpool.tile([128, 256], BF16)
            dmaA(vb[:], vb_d[:])
            w1fpbd = wpool.tile([128, 64], BF16)
            dmaA(w1fpbd[:], w1fpbd_d[:])
            w2pk = wpool.tile([128, 256], BF16)
            dmaA(w2pk[:], w2pk_d[:])

            # x-side gathers (overlap phases A/B)
            xg = [wpool.tile([128, 128], F32, name=f"xg{t}")
                  for t in range(8)]
            for t in range(8):
                nc.gpsimd.indirect_copy(
                    v(xg[t], 0, [[xg[t].ap[0][0], 128], [1, 128], [1, 1]]),
                    dx[:], ixX[:, t * 8:(t + 1) * 8], True)

            # w1 summed-tap images -> w1stp4[g] [128(k2x,m), 100]
            w1s4 = wpool.tile([128, 16], F32)
            nc.vector.tensor_reduce(
                w1s4[:],
                v(w1sb4, 0, [[w1sb4.ap[0][0], 128], [1, 16], [16, 3]]),
                AX.X, ALU.add)
            w1si = wpool.tile([128, 256], F32)
            nc.vector.memset(w1si[:], 0.0)
            nc.vector.tensor_copy(
                v(w1si, 102, [[w1si.ap[0][0], 128], [16, 4], [1, 4]]),
                w1s4[:])
            w1stp4 = [wpool.tile([128, 100], F32, name=f"w1stp4{g}")
                      for g in range(4)]
            wpitch = w1si.ap[0][0]
            for g in range(4):
                for k2x in range(4):
                    dst = w1stp4[g]
                    nc.vector.tensor_copy(
                        v(dst, k2x * 32 * dst.ap[0][0],
                          [[dst.ap[0][0], 32], [10, 10], [1, 10]]),
                        v(w1si, k2x * 32 * wpitch
                          + (6 - 2 * g) * 16 + 6 - 2 * k2x,
                          [[wpitch, 32], [16, 10], [1, 10]]))

            # ---- Phase A: conv1 (one matmul) ----
            y1ps = psA.tile([128, 512], F32, tag="psA", name="y1ps")
            nc.tensor.matmul(
                y1ps[:], w1f4[:],
                v(X48, 78, [[X48.ap[0][0], 48], [1444, 2], [76, 16],
                            [2, 16]]),
                start=True, stop=True)
            y1sb = wpool.tile([128, 512], F32)
            nc.scalar.activation(y1sb[:], y1ps[:], AF.Relu, bias=b1t4[:])

            y1p4 = wpool.tile([128, 648], F32)
            nc.vector.memset(y1p4[:], 0.0)
            ypitch = y1p4.ap[0][0]
            nc.vector.tensor_copy(
                v(y1p4, 19, [[ypitch, 128], [324, 2], [18, 16], [1, 16]]),
                v(y1sb, 0,
                  [[y1sb.ap[0][0], 128], [256, 2], [16, 16], [1, 16]]))
            m1p4 = wpool.tile([128, 648], F32)
            nc.vector.tensor_scalar(m1p4[:], y1p4[:], 0.0, None, ALU.is_gt)

            # shifted-row copy for conv2 im2col (block k2x shifted by k2x)
            Y2 = wpool.tile([128, 648], F32)
            nc.vector.tensor_copy(Y2[0:32, :], y1p4[0:32, :])
            nc.vector.tensor_copy(Y2[32:64, 0:647], y1p4[32:64, 1:648])
            nc.scalar.copy(Y2[64:96, 0:646], y1p4[64:96, 2:648])
            nc.scalar.copy(Y2[96:128, 0:645], y1p4[96:128, 3:648])

            def tapv(tl, pbase, k2, pitch, n=32):
                k2y, k2x = k2 // 4, k2 % 4
                return v(tl, pbase * pitch + 18 * k2y + k2x,
                         [[pitch, n], [324, 2], [36, 8], [2, 8]])

            # ---- conv2: 4 matmuls ----
            ypre = psA.tile([64, 128], F32, tag="psA", name="ypre")
            for k2y in range(4):
                nc.tensor.matmul(
                    ypre[:],
                    v(W2p, k2y * 64, [[W2p.ap[0][0], 128], [1, 64]]),
                    v(Y2, 18 * k2y,
                      [[Y2.ap[0][0], 128], [324, 2], [36, 8], [2, 8]]),
                    start=(k2y == 0), stop=(k2y == 3))
            yT = wpool.tile([64, 128], F32)
            nc.scalar.activation(yT[:], ypre[:], AF.Relu, bias=b2t[:])
            m2T = wpool.tile([64, 128], F32)
            nc.vector.tensor_scalar(m2T[:], yT[:], 0.0, None, ALU.is_gt)

            # m2 mask in (b,o)-partition layout for phase C
            yTT_ps = psB.tile([128, 64], F32, tag="psB", name="yTT_ps")
            nc.tensor.transpose(yTT_ps[:], yT[:], ident[0:64, 0:64])
            m2g = wpool.tile([128, 64], F32)
            nc.vector.tensor_scalar(m2g[:], yTT_ps[:], 0.0, None, ALU.is_gt)

            def hopfield_f32(src, tag):
                a_ps = psA.tile([128, 512], F32, tag="psA",
                                name=f"a_ps{tag}")
                nc.tensor.matmul(a_ps[:], src, ktK[:], start=True,
                                 stop=True)
                rmax = wpool.tile([128, 1], F32, name=f"rmax{tag}")
                nc.vector.tensor_reduce(rmax[:], a_ps[:], AX.X, ALU.max)
                negbm = wpool.tile([128, 1], F32, name=f"negbm{tag}")
                nc.vector.tensor_scalar(negbm[:], rmax[:], -0.125, None,
                                        ALU.mult)
                p_sb = wpool.tile([128, 512], F32, name=f"p_sb{tag}")
                ssum = wpool.tile([128, 1], F32, name=f"ssum{tag}")
                nc.scalar.activation(p_sb[:], a_ps[:], AF.Exp,
                                     bias=negbm[:], scale=0.125,
                                     accum_out=ssum[:])
                rec = wpool.tile([128, 1], F32, name=f"rec{tag}")
                nc.vector.reciprocal(rec[:], ssum[:])
                nc.vector.tensor_scalar(p_sb[:], p_sb[:], rec[:], None,
                                        ALU.mult)
                yq_ps = psC.tile([128, 64], F32, tag="psC",
                                 name=f"yq_ps{tag}")
                for t in range(4):
                    pt_ps = psB.tile([128, 128], F32, tag="psB",
                                     name=f"pt_ps{tag}{t}")
                    nc.tensor.transpose(pt_ps[:],
                                        p_sb[:, t * 128:(t + 1) * 128],
                                        ident[:])
                    pt_sb = wpool.tile([128, 128], F32, tag="pt_sb",
                                       name=f"pt_sb{tag}{t}")
                    nc.scalar.copy(pt_sb[:], pt_ps[:])
                    nc.tensor.matmul(yq_ps[:], pt_sb[:],
                                     vV[:, t * 64:(t + 1) * 64],
                                     start=(t == 0), stop=(t == 3))
                yq_sb = wpool.tile([128, 64], F32, name=f"yq_sb{tag}")
                nc.scalar.copy(yq_sb[:], yq_ps[:])
                return yq_sb

            yq1 = hopfield_f32(yT[:], "h1")

            yqT_ps = psB.tile([64, 128], F32, tag="psB", name="yqT_ps")
            nc.tensor.transpose(yqT_ps[:], yq1[:], ident[:])
            r2T = wpool.tile([64, 128], F32)
            nc.vector.scalar_tensor_tensor(r2T[:], yqT_ps[:], -1.0, yT[:],
                                           ALU.mult, ALU.add)
            nc.vector.tensor_mul(r2T[:], r2T[:], m2T[:])

            # ---- Phase B: g1 pairs + ep ----
            g1m4 = [wpool.tile([128, 128], F32, name=f"g1m4{g}")
                    for g in range(4)]
            for g in range(4):
                g1ps = psB.tile([128, 128], F32, tag="psB",
                                name=f"g1ps{g}")
                for j in range(2):
                    nc.tensor.matmul(
                        g1ps[64 * j:64 * j + 64, :],
                        v(w2k2m, (2 * g + j) * 64,
                          [[w2k2m.ap[0][0], 64], [1, 64]]),
                        r2T[:], start=True, stop=True)
                for k2l in range(4):
                    nc.vector.tensor_tensor(
                        g1m4[g][k2l * 32:(k2l + 1) * 32, :],
                        g1ps[k2l * 32:(k2l + 1) * 32, :],
                        tapv(m1p4, k2l * 32, 4 * g + k2l, ypitch),
                        ALU.mult)

            ep_ps = psA.tile([100, 128], F32, tag="psA", name="ep_ps")
            for g in range(4):
                nc.tensor.matmul(ep_ps[:], w1stp4[g][:], g1m4[g][:],
                                 start=(g == 0), stop=(g == 3))
            ep_sb = wpool.tile([100, 128], F32)
            nc.scalar.copy(ep_sb[:], ep_ps[:])
            ep2_ps = psB.tile([128, 100], F32, tag="psB", name="ep2_ps")
            nc.tensor.transpose(ep2_ps[:], ep_sb[:], ident[0:100, 0:100])
            ep2 = wpool.tile([128, 100], F32)
            nc.scalar.copy(ep2[:], ep2_ps[:])

            # scatter e-patches to DRAM (single 4-dim DMA)
            for b, dq in ((0, dmaS), (1, dmaA)):
                dq(v(ed4p, b * 14400 + 2 * 1200 + 2 * 100,
                     [[1200, 8], [100, 8], [1, 100]]),
                   ep2[b * 64:(b + 1) * 64, :])

            # data_e (dedup: two distinct 16-row contents + 2 copies)
            de = wpool.tile([128, 3600], F32)
            for r, dq in ((0, dmaS), (1, dmaA), (2, dmaS), (3, dmaA)):
                dq(de[r * 32:r * 32 + 16, :],
                   v(ed4p, 1200 * (1 if r >= 1 else 0),
                     [[14400, 2], [1200, 8], [1, 3600]]))
            for r, dq in ((0, dmaA), (1, dmaS), (2, dmaA), (3, dmaS)):
                dq(de[r * 32 + 16:r * 32 + 32, :],
                   de[r * 32:r * 32 + 16, :])

            e9 = wpool.tile([128, 144], F32)
            e9p = e9.ap[0][0]
            nc.gpsimd.indirect_copy(
                v(e9, 0, [[e9p, 128], [1, 144], [1, 1]]),
                de[:], idxE[:], True)

            # argmin with reference tie semantics (v1 128-row layout)
            mincand = wpool.tile([128, 16], F32)
            nc.vector.tensor_reduce(
                mincand[:], v(e9, 0, [[e9p, 128], [9, 16], [1, 9]]),
                AX.X, ALU.min)
            mstar = wpool.tile([128, 16], F32)
            nc.vector.tensor_scalar(mstar[:], mincand[:], 0.0, None,
                                    ALU.min)
            eq9 = wpool.tile([128, 144], F32)
            nc.vector.tensor_tensor(
                v(eq9, 0, [[eq9.ap[0][0], 128], [9, 16], [1, 9]]),
                v(e9, 0, [[e9p, 128], [9, 16], [1, 9]]),
                v(mstar, 0, [[mstar.ap[0][0], 128], [1, 16], [0, 9]]),
                ALU.is_equal)
            cs = wpool.tile([128, 144], F32)
            nc.vector.scalar_tensor_tensor(cs[:], eq9[:], -1000.0,
                                           oidx9[:], ALU.mult, ALU.add)
            minc2 = wpool.tile([128, 16], F32)
            nc.vector.tensor_reduce(
                minc2[:], v(cs, 0, [[cs.ap[0][0], 128], [9, 16], [1, 9]]),
                AX.X, ALU.min)
            zeq = wpool.tile([128, 16], F32)
            nc.vector.tensor_scalar(zeq[:], mstar[:], 0.0, None,
                                    ALU.is_equal)
            zsc = wpool.tile([128, 16], F32)
            nc.vector.scalar_tensor_tensor(zsc[:], zeq[:], -1000.0,
                                           zc128[:], ALU.mult, ALU.add)
            sel16 = wpool.tile([128, 16], F32)
            nc.vector.tensor_tensor(sel16[:], minc2[:], zsc[:], ALU.min)
            nc.vector.tensor_scalar(sel16[:], sel16[:], 1000.0, None,
                                    ALU.add)

            # permute rows (r,h,b,q)->(b,iy) x ix-halves, ONE 3-dim scatter
            selbi_ps = psC.tile([64, 32], F32, tag="psC", name="selbi_ps")
            nc.tensor.matmul(selbi_ps[:, 0:16], permA[:], sel16[:],
                             start=True, stop=True)
            nc.tensor.matmul(selbi_ps[:, 16:32], permB[:], sel16[:],
                             start=True, stop=True)
            selbi = wpool.tile([64, 32], F32)
            nc.scalar.copy(selbi[:], selbi_ps[:])
            dmaS(v(sel_pad, 117, [[1444, 2], [38, 32], [1, 32]]),
                 v(selbi, 0, [[selbi.ap[0][0], 64], [1, 32]]))

            # sel image rows with (k1y,k1x) shifts, replicated x8
            ds = wpool.tile([128, 2888], F32)
            qs = (dmaS, dmaA, dmaS, dmaA, dmaS, dmaA, dmaS, dmaA)
            for k in range(8):
                qs[k](ds[k * 16:(k + 1) * 16, :],
                      v(sel_pad, 0, [[38, 4], [1, 4], [1, 2888]]))

            selm2 = []
            for s in range(2):
                sg = wpool.tile([128, 128], F32, name=f"sg{s}")
                nc.gpsimd.indirect_copy(
                    v(sg, 0, [[sg.ap[0][0], 128], [1, 128], [1, 1]]),
                    ds[:], ixS[:, s * 8:(s + 1) * 8], True)
                sgb = wpool.tile([128, 128], BF16, name=f"sgb{s}")
                nc.vector.tensor_tensor(sgb[:], sg[:], oidx128[:],
                                        ALU.is_equal)
                selm2.append(sgb)

            # ---- Phase C (bf16): masks -> xsel -> z pairs -> ym ----
            xsel = []
            for t in range(8):
                mx_ps = psB.tile([128, 128], F32, tag="psB",
                                 name=f"mx{t}")
                nc.tensor.matmul(mx_ps[:],
                                 emt8[:, t * 128:(t + 1) * 128],
                                 selm2[(t // 2) // 2][:],
                                 start=True, stop=True)
                xs = wpool.tile([128, 128], BF16, name=f"xs{t}")
                nc.vector.tensor_tensor(xs[:], xg[t][:], mx_ps[:],
                                        ALU.mult)
                xsel.append(xs)

            zm4 = [wpool.tile([128, 128], BF16, name=f"zm4{g}")
                   for g in range(4)]
            for g in range(4):
                zps = psB.tile([128, 128], F32, tag="psB", name=f"zps{g}")
                for k2xh in range(2):
                    t = g * 2 + k2xh
                    nc.tensor.matmul(zps[64 * k2xh:64 * k2xh + 64, :],
                                     w1fpbd[:], xsel[t][:],
                                     start=True, stop=True)
                for k2x in range(4):
                    nc.vector.tensor_tensor(
                        zm4[g][k2x * 32:(k2x + 1) * 32, :],
                        zps[k2x * 32:(k2x + 1) * 32, :],
                        tapv(m1p4, k2x * 32, 4 * g + k2x, ypitch),
                        ALU.mult)

            ym_ps = psC.tile([128, 64], F32, tag="psC", name="ym_ps")
            for g in range(4):
                nc.tensor.matmul(ym_ps[:], zm4[g][:],
                                 w2pk[:, g * 64:(g + 1) * 64],
                                 start=(g == 0), stop=(g == 3))
            ymm = wpool.tile([128, 64], BF16)
            nc.vector.tensor_tensor(ymm[:], ym_ps[:], m2g[:], ALU.mult)

            t2_ps = psB.tile([64, 128], BF16, tag="psB", name="t2_ps")
            nc.tensor.transpose(t2_ps[:], ymm[:], identb[:])
            ymmT = wpool.tile([64, 128], BF16)
            nc.scalar.copy(ymmT[:], t2_ps[:])

            # hopfield 2 in bf16
            a_ps = psA.tile([128, 512], F32, tag="psA", name="a_ps2")
            nc.tensor.matmul(a_ps[:], ymmT[:], ktb[:], start=True,
                             stop=True)
            rmax = wpool.tile([128, 1], F32, name="rmax2")
            nc.vector.tensor_reduce(rmax[:], a_ps[:], AX.X, ALU.max)
            negbm = wpool.tile([128, 1], F32, name="negbm2")
            nc.vector.tensor_scalar(negbm[:], rmax[:], -0.125, None,
                                    ALU.mult)
            p_sb = wpool.tile([128, 512], BF16, name="p_sb2")
            ssum = wpool.tile([128, 1], F32, name="ssum2")
            nc.scalar.activation(p_sb[:], a_ps[:], AF.Exp, bias=negbm[:],
                                 scale=0.125, accum_out=ssum[:])
            rec = wpool.tile([128, 1], F32, name="rec2")
            nc.vector.reciprocal(rec[:], ssum[:])
            nc.vector.tensor_scalar(p_sb[:], p_sb[:], rec[:], None,
                                    ALU.mult)
            yq2_ps = psC.tile([128, 64], F32, tag="psC", name="yq2_ps")
            for t in range(4):
                pt_ps = psB.tile([128, 128], BF16, tag="psB",
                                 name=f"pt2_{t}")
                nc.tensor.transpose(pt_ps[:],
                                    p_sb[:, t * 128:(t + 1) * 128],
                                    identb[:])
                pt_sb = wpool.tile([128, 128], BF16, tag="pt_sb2",
                                   name=f"pt_sb2{t}")
                nc.scalar.copy(pt_sb[:], pt_ps[:])
                nc.tensor.matmul(yq2_ps[:], pt_sb[:],
                                 vb[:, t * 64:(t + 1) * 64],
                                 start=(t == 0), stop=(t == 3))
            yq2 = wpool.tile([128, 64], F32)
            nc.scalar.copy(yq2[:], yq2_ps[:])

            tr_ps = psB.tile([64, 128], F32, tag="psB", name="tr_ps")
            nc.tensor.transpose(tr_ps[:], yq2[:], ident[:])
            outT = wpool.tile([64, 128], F32)
            nc.scalar.copy(outT[:], tr_ps[:])
            dmaS(AP(out_d, 0, [[64, 64], [4096, 2], [8, 8], [1, 8]]),
                 v(outT, 0, [[outT.ap[0][0], 64], [1, 128]]))

    return nc


_CACHE = {}


def kernel(**inputs) -> np.ndarray:
    from concourse.bass_utils import run_bass_kernel_spmd
    if "nc" not in _CACHE:
        from concourse import bacc
        nc = bacc.Bacc("TRN2", target_bir_lowering=False, debug=False,
                       num_devices=N_CORES)
        build_program(nc)
        nc.compile()
        _CACHE["nc"] = nc
    nc = _CACHE["nc"]
    feed = prepare_feed(inputs)
    in_maps = [dict(feed) for _ in range(N_CORES)]
    res = run_bass_kernel_spmd(nc, in_maps, list(range(N_CORES)))
    return np.asarray(res.results[0]["out"], np.float32)


# revision 11
# speedup vs baseline: 1.8516x; 1.0064x over previous
"""Trainium2 Bass kernel for nn_Block1_87144886436577 (vq_codebook), v2.

Analytic collapse of the reference's jacobians (see v1 docstring), with:
- conv1 as ONE fp32 matmul (im2col rows (ci,k1y,k1x) built by 3 DMAs
  from the host-padded image, replicated 4x in partition blocks),
- conv2 as 4 fp32 matmuls contracting (k2x, m)=128 per k2y over a
  shifted-row copy of y1,
- phase-B backprop (g1/ep) packed into 8+4 fp32 matmuls via
  host-rearranged weights; fp32 is REQUIRED upstream of the argmin
  (sel margins are ~1e-3; bf16 flips selections),
- the e-patch/sel DRAM round-trips deduplicated + spread across the
  sync/scalar/gpsimd DMA queues, argmin on a 64-partition layout, a
  permutation matmul so sel scatters with a single 3-dim DMA,
- everything downstream of sel (mask expansion, masked conv taps,
  second hopfield) in bf16 with tap-pair packing,
- all weight repacks/transposes done host-side (pure layout + casts).

All 8 cores run identical replicas; output read from core 0.
"""
import sys

import numpy as np

for _p in ("/opt/trn_rl_repo",):
    if _p not in sys.path:
        sys.path.insert(0, _p)

import concourse.bass as bass
import concourse.mybir as mybir
import concourse.tile as tile

F32 = mybir.dt.float32
BF16 = mybir.dt.bfloat16
U16 = mybir.dt.uint16
AF = mybir.ActivationFunctionType
ALU = mybir.AluOpType
AX = mybir.AxisListType
AP = bass.AP

N_CORES = 8


def v(t, off, pat):
    """Custom-view AP over a tile (t = AP returned by pool.tile)."""
    return AP(t.tensor, t.offset + off, pat)


def _e(r):
    return 1 if r >= 1 else 0


def _tables():
    """Input-independent index/mask tables."""
    import ml_dtypes
    BF = ml_dtypes.bfloat16

    ident = np.eye(128, dtype=np.float32)
    identb = ident.astype(BF)
    oidx128 = np.tile((np.arange(128) % 64).astype(BF)[None, :],
                      (128, 1))

    # xsel gather streams (same as v1): tile t=(k2y,k2xh); partition
    # p=k2xp*64+k1y*16+k1x*4+ci; j<128 per tile: (b,oy,ox).
    idxX = np.zeros((8, 128, 8), np.uint16)
    for t in range(8):
        k2y, k2xh = t // 2, t % 2
        for g in range(8):
            k2xp = g // 4
            k1y = g % 4
            k2x = 2 * k2xh + k2xp
            for j in range(128):
                b, oy, ox = j // 64, (j % 64) // 8, j % 8
                idxX[t, 16 * g + j % 16, j // 16] = (
                    b * 1444 + (4 * oy + 2 * k2y + k1y) * 38
                    + 4 * ox + 2 * k2x)
    ixX = np.zeros((128, 64), np.uint16)
    for t in range(8):
        ixX[:, t * 8:(t + 1) * 8] = idxX[t]

    # sel gather streams (as v1)
    idxS = np.zeros((2, 128, 8), np.uint16)
    for s in range(2):
        for g in range(8):
            k2yp, k2x = g // 4, g % 4
            k2y = s * 2 + k2yp
            for j in range(128):
                b, oy, ox = j // 64, (j % 64) // 8, j % 8
                idxS[s, 16 * g + j % 16, j // 16] = (
                    b * 1444 + (4 * oy + 2 * k2y) * 38 + 4 * ox + 2 * k2x)
    ixS = np.zeros((128, 16), np.uint16)
    for s in range(2):
        ixS[:, s * 8:(s + 1) * 8] = idxS[s]

    # mask expansion matrices, bf16: emt8[r, t*128+p]
    emt8 = np.zeros((128, 1024), np.float32)
    for t in range(8):
        k2y, k2xh = t // 2, t % 2
        for p in range(128):
            k2xp, k1y, k1x = p // 64, (p % 64) // 16, p % 4
            k2x = 2 * k2xh + k2xp
            r = (k2y % 2) * 64 + k2x * 16 + k1y * 4 + k1x
            emt8[r, t * 128 + p] = 1.0
    emt8 = emt8.astype(BF)

    # E9 gather (v1 layout): p = r*32+h*16+b*8+q (iy=4q+r, ix=16h+ixl),
    # stream j = ixl*9 + jj. data row = ed4p[b, q+e(r) : +3 rows] flat.
    idxE = np.zeros((128, 9), np.uint16)
    oidx9 = np.full((128, 144), 3000.0, np.float32)
    zc128 = np.zeros((128, 16), np.float32)
    for r in range(4):
        for h in range(2):
            g = r * 2 + h
            for j in range(144):
                ixl, jj = j // 9, j % 9
                jy, jx = jj // 3, jj % 3
                t_ = ixl % 4
                s = 4 * h + ixl // 4
                dy = r - 4 * _e(r) + 4 * jy + 3
                dx = t_ - 4 * _e(t_) + 4 * jx + 3
                ox = s + _e(t_) - jx
                if 0 <= dy < 10 and 0 <= dx < 10:
                    idx = (2 - jy) * 1200 + (ox + 2) * 100 + dy * 10 + dx
                else:
                    idx = 0  # guaranteed-zero pad cell
                idxE[16 * g + j % 16, j // 16] = idx
    for r in range(4):
        for h in range(2):
            for b in range(2):
                for q in range(8):
                    p = r * 32 + h * 16 + b * 8 + q
                    iy = 4 * q + r
                    for ixl in range(16):
                        ix = 16 * h + ixl
                        t_ = ix % 4
                        s = ix // 4
                        for jj in range(9):
                            jy, jx = jj // 3, jj % 3
                            oy = q + _e(r) - jy
                            ox = s + _e(t_) - jx
                            dy = iy - 4 * oy + 3
                            dx = ix - 4 * ox + 3
                            if (0 <= oy < 8 and 0 <= ox < 8
                                    and 0 <= dy < 10 and 0 <= dx < 10):
                                oidx9[p, ixl * 9 + jj] = oy * 8 + ox
                        for o in range(64):
                            oy, ox = o // 8, o % 8
                            if not (0 <= iy - 4 * oy + 3 < 10
                                    and 0 <= ix - 4 * ox + 3 < 10):
                                zc128[p, ixl] = float(o)
                                break

    # permutations: sel16 rows (r,h,b,q) -> rows (b, iy), cols ix-halves
    permA = np.zeros((128, 64), np.float32)
    permB = np.zeros((128, 64), np.float32)
    for r in range(4):
        for h in range(2):
            for b in range(2):
                for q in range(8):
                    p = r * 32 + h * 16 + b * 8 + q
                    (permA if h == 0 else permB)[p, b * 32 + 4 * q + r] \
                        = 1.0

    neg1s = np.full((2, 1520), -1.0, BF)
    zed = np.zeros((12, 2400), np.float32)
    return {"ident": ident, "identb": identb, "oidx128": oidx128,
            "ixX": ixX, "ixS": ixS, "emt8": emt8, "idxE": idxE,
            "oidx9": oidx9, "zc128": zc128, "permA": permA,
            "permB": permB, "neg1s": neg1s, "zed": zed}


def prepare_feed(inputs):
    """Host-side layout/cast-only rearrangements of the inputs."""
    import ml_dtypes
    BF = ml_dtypes.bfloat16
    x = np.asarray(inputs["x"], np.float32)    # (2,3,32,32)
    w1 = np.asarray(inputs["w1"], np.float32)  # (32,3,4,4)
    b1 = np.asarray(inputs["b1"], np.float32)  # (32,)
    w2 = np.asarray(inputs["w2"], np.float32)  # (64,32,4,4)
    b2 = np.asarray(inputs["b2"], np.float32)  # (64,)
    K = np.asarray(inputs["K"], np.float32)    # (512,64)
    V = np.asarray(inputs["V"], np.float32)    # (512,64)

    f = dict(_TABLES)

    xpad = np.zeros((3, 3008), np.float32)
    img = np.zeros((3, 2, 38, 38), np.float32)
    img[:, :, 3:35, 3:35] = x.transpose(1, 0, 2, 3)
    xpad[:, 0:2888] = img.reshape(3, 2888)
    f["xpad3"] = xpad

    # data_x rows (ci,k1x): padded image shifted left by k1x; ci=3 zero.
    dx16 = np.zeros((16, 2888), np.float32)
    for ci in range(3):
        for k1x in range(4):
            dx16[ci * 4 + k1x, 0:2888 - k1x] = \
                f["xpad3"][ci, k1x:2888]
    f["dx16"] = dx16.astype(BF)

    # conv1 weights: [ci*16+k1y*4+k1x, j*32+m], 4 dup col blocks
    w1t = w1.transpose(1, 2, 3, 0).reshape(48, 32)
    f["w1f4"] = np.tile(w1t, (1, 4)).copy()
    f["b1t4"] = np.tile(b1, 4).reshape(128, 1).copy()

    # conv2 weights: [k2x*32+m, k2y*64+c]
    W2p = np.zeros((128, 256), np.float32)
    for k2y in range(4):
        for k2x in range(4):
            W2p[k2x * 32:(k2x + 1) * 32, k2y * 64:(k2y + 1) * 64] = \
                w2[:, :, k2y, k2x].T
    f["W2p"] = W2p
    f["b2t"] = b2.reshape(64, 1).copy()

    f["ktK"] = K.T.copy()                       # [64, 512]
    f["vV"] = V.reshape(4, 128, 64).transpose(1, 0, 2).reshape(128, 256) \
        .copy()                                 # [128, (t,c)]
    f["ktb"] = f["ktK"].astype(BF)
    f["vb"] = f["vV"].astype(BF)

    # g1 pair weights: [c, k2*32+m]
    w2k2m = np.zeros((64, 512), np.float32)
    for k2 in range(16):
        k2y, k2x = k2 // 4, k2 % 4
        w2k2m[:, k2 * 32:(k2 + 1) * 32] = w2[:, :, k2y, k2x]
    f["w2k2m"] = w2k2m

    # w1 summed-tap images, built on chip from w1sb4
    f["w1sb4"] = np.tile(w1.reshape(32, 48), (4, 1)).copy()

    # z-pair blockdiag weights bf16 [128, 64]
    w1fp64 = np.zeros((64, 32), np.float32)
    for k1y in range(4):
        for ci in range(3):
            for k1x in range(4):
                w1fp64[k1y * 16 + ci * 4 + k1x, :] = w1[:, ci, k1y, k1x]
    bd = np.zeros((128, 64), np.float32)
    bd[0:64, 0:32] = w1fp64
    bd[64:128, 32:64] = w1fp64
    f["w1fpbd"] = bd.astype(BF)

    # ym pack weights bf16 [128, (g,c)]
    w2pk = np.zeros((128, 256), np.float32)
    for g in range(4):
        for k2x in range(4):
            w2pk[k2x * 32:(k2x + 1) * 32, g * 64:(g + 1) * 64] = \
                w2[:, :, g, k2x].T
    f["w2pk"] = w2pk.astype(BF)
    return f


_TABLES = _tables()


def build_program(nc):
    def P(name, shape, dt):
        return nc.declare_dram_parameter(name, shape, dt, isOutput=False)

    xpad_d = P("xpad3", [3, 3008], F32)
    dx16_d = P("dx16", [16, 2888], BF16)
    w1f4_d = P("w1f4", [48, 128], F32)
    b1t4_d = P("b1t4", [128, 1], F32)
    W2p_d = P("W2p", [128, 256], F32)
    b2t_d = P("b2t", [64, 1], F32)
    ktK_d = P("ktK", [64, 512], F32)
    vV_d = P("vV", [128, 256], F32)
    ktb_d = P("ktb", [64, 512], BF16)
    vb_d = P("vb", [128, 256], BF16)
    w2k2m_d = P("w2k2m", [64, 512], F32)
    w1sb4_d = P("w1sb4", [128, 48], F32)
    w1fpbd_d = P("w1fpbd", [128, 64], BF16)
    w2pk_d = P("w2pk", [128, 256], BF16)
    ident_d = P("ident", [128, 128], F32)
    identb_d = P("identb", [128, 128], BF16)
    oidx128_d = P("oidx128", [128, 128], BF16)
    ixX_d = P("ixX", [128, 64], U16)
    ixS_d = P("ixS", [128, 16], U16)
    emt8_d = P("emt8", [128, 1024], BF16)
    idxE_d = P("idxE", [128, 9], U16)
    oidx9_d = P("oidx9", [128, 144], F32)
    zc128_d = P("zc128", [128, 16], F32)
    permA_d = P("permA", [128, 64], F32)
    permB_d = P("permB", [128, 64], F32)
    neg1s_d = P("neg1s", [2, 1520], BF16)
    zed_d = P("zed", [12, 2400], F32)
    out_d = nc.declare_dram_parameter("out", [2, 64, 8, 8], F32,
                                      isOutput=True)

    with tile.TileContext(nc) as tc:
        with (
            tc.tile_pool(name="const", bufs=1) as cpool,
            tc.tile_pool(name="work", bufs=1) as wpool,
            tc.tile_pool(name="psA", bufs=2, space="PSUM") as psA,
            tc.tile_pool(name="psB", bufs=4, space="PSUM") as psB,
            tc.tile_pool(name="psC", bufs=2, space="PSUM") as psC,
            tc.tile_pool(name="dram", bufs=1, space="DRAM") as dpool,
        ):
            dmaS = nc.sync.dma_start
            dmaA = nc.scalar.dma_start
            dmaG = nc.gpsimd.dma_start

            # ---- DRAM scratch ----
            ed4p = dpool.tile([2, 12, 12, 100], F32)
            sel_pad = dpool.tile([3040], BF16)

            # ---- critical-path loads first (sync queue) ----
            w1f4 = wpool.tile([48, 128], F32)
            dmaS(w1f4[:], w1f4_d[:])
            X48 = wpool.tile([48, 2888], F32)
            for ci, dq in ((0, dmaS), (1, dmaA), (2, dmaS)):
                dq(X48[ci * 16:(ci + 1) * 16, :],
                   AP(xpad_d, ci * 3008, [[38, 4], [1, 4], [1, 2888]]))
            b1t4 = wpool.tile([128, 1], F32)
            dmaS(b1t4[:], b1t4_d[:])
            W2p = wpool.tile([128, 256], F32)
            dmaS(W2p[:], W2p_d[:])
            b2t = wpool.tile([64, 1], F32)
            dmaS(b2t[:], b2t_d[:])
            ktK = wpool.tile([64, 512], F32)
            dmaS(ktK[:], ktK_d[:])
            ident = cpool.tile([128, 128], F32)
            dmaS(ident[:], ident_d[:])
            vV = wpool.tile([128, 256], F32)
            dmaS(vV[:], vV_d[:])
            w2k2m = wpool.tile([64, 512], F32)
            dmaA(w2k2m[:], w2k2m_d[:])
            w1sb4 = wpool.tile([128, 48], F32)
            dmaA(w1sb4[:], w1sb4_d[:])

            # ---- non-critical loads (scalar/gpsimd queues) ----
            dmaA(v(ed4p, 0, [[2400, 12], [1, 2400]]), zed_d[:])
            dmaA(v(sel_pad, 0, [[1520, 2], [1, 1520]]), neg1s_d[:])
            ixX = cpool.tile([128, 64], U16)
            dmaS(ixX[:], ixX_d[:])
            dx = wpool.tile([128, 2888], BF16)
            dmaS(dx[0:16, :], dx16_d[:])
            for d in (16, 32, 64):
                dmaS(dx[d:2 * d, :], dx[0:d, :])
            identb = cpool.tile([128, 128], BF16)
            dmaA(identb[:], identb_d[:])
            oidx128 = cpool.tile([128, 128], BF16)
            dmaA(oidx128[:], oidx128_d[:])
            ixS = cpool.tile([128, 16], U16)
            dmaA(ixS[:], ixS_d[:])
            emt8 = cpool.tile([128, 1024], BF16)
            dmaA(emt8[:], emt8_d[:])
            idxE = cpool.tile([128, 9], U16)
            dmaA(idxE[:], idxE_d[:])
            oidx9 = cpool.tile([128, 144], F32)
            dmaA(oidx9[:], oidx9_d[:])
            zc128 = cpool.tile([128, 16], F32)
            dmaA(zc128[:], zc128_d[:])
            permA = cpool.tile([128, 64], F32)
            dmaA(permA[:], permA_d[:])
            permB = cpool.tile([128, 64], F32)
            dmaA(permB[:], permB_d[:])
            ktb = wpool.tile([64, 512], BF16)
            dmaA(ktb[:], ktb_d[:])
            vb = wpool.tile([128, 256], BF16)
            dmaA(vb[:], vb_d[:])
            w1fpbd = wpool.tile([128, 64], BF16)
            dmaA(w1fpbd[:], w1fpbd_d[:])
            w2pk = wpool.tile([128, 256], BF16)
            dmaA(w2pk[:], w2pk_d[:])

            # x-side gathers (overlap phases A/B)
            xg = [wpool.tile([128, 128], BF16, name=f"xg{t}")
                  for t in range(8)]
            for t in range(8):
                nc.gpsimd.indirect_copy(
                    v(xg[t], 0, [[xg[t].ap[0][0], 128], [1, 128], [1, 1]]),
                    dx[:], ixX[:, t * 8:(t + 1) * 8], True)

            # w1 summed-tap images -> w1stp4[g] [128(k2x,m), 100]
            w1s4 = wpool.tile([128, 16], F32)
            nc.vector.tensor_reduce(
                w1s4[:],
                v(w1sb4, 0, [[w1sb4.ap[0][0], 128], [1, 16], [16, 3]]),
                AX.X, ALU.add)
            w1si = wpool.tile([128, 256], F32)
            nc.vector.memset(w1si[:], 0.0)
            nc.vector.tensor_copy(
                v(w1si, 102, [[w1si.ap[0][0], 128], [16, 4], [1, 4]]),
                w1s4[:])
            w1stp4 = [wpool.tile([128, 100], F32, name=f"w1stp4{g}")
                      for g in range(4)]
            wpitch = w1si.ap[0][0]
            for g in range(4):
                for k2x in range(4):
                    dst = w1stp4[g]
                    nc.vector.tensor_copy(
                        v(dst, k2x * 32 * dst.ap[0][0],
                          [[dst.ap[0][0], 32], [10, 10], [1, 10]]),
                        v(w1si, k2x * 32 * wpitch
                          + (6 - 2 * g) * 16 + 6 - 2 * k2x,
                          [[wpitch, 32], [16, 10], [1, 10]]))

            # ---- Phase A: conv1 (one matmul) ----
            y1ps = psA.tile([128, 512], F32, tag="psA", name="y1ps")
            nc.tensor.matmul(
                y1ps[:], w1f4[:],
                v(X48, 78, [[X48.ap[0][0], 48], [1444, 2], [76, 16],
                            [2, 16]]),
                start=True, stop=True)
            y1sb = wpool.tile([128, 512], F32)
            nc.scalar.activation(y1sb[:], y1ps[:], AF.Relu, bias=b1t4[:])

            y1p4 = wpool.tile([128, 648], F32)
            nc.vector.memset(y1p4[:], 0.0)
            ypitch = y1p4.ap[0][0]
            nc.vector.tensor_copy(
                v(y1p4, 19, [[ypitch, 128], [324, 2], [18, 16], [1, 16]]),
                v(y1sb, 0,
                  [[y1sb.ap[0][0], 128], [256, 2], [16, 16], [1, 16]]))
            m1p4 = wpool.tile([128, 648], F32)
            nc.vector.tensor_scalar(m1p4[:], y1p4[:], 0.0, None, ALU.is_gt)

            # shifted-row copy for conv2 im2col (block k2x shifted by k2x)
            Y2 = wpool.tile([128, 648], F32)
            nc.vector.tensor_copy(Y2[0:32, :], y1p4[0:32, :])
            nc.vector.tensor_copy(Y2[32:64, 0:647], y1p4[32:64, 1:648])
            nc.scalar.copy(Y2[64:96, 0:646], y1p4[64:96, 2:648])
            nc.scalar.copy(Y2[96:128, 0:645], y1p4[96:128, 3:648])

            def tapv(tl, pbase, k2, pitch, n=32):
                k2y, k2x = k2 // 4, k2 % 4
                return v(tl, pbase * pitch + 18 * k2y + k2x,
                         [[pitch, n], [324, 2], [36, 8], [2, 8]])

            # ---- conv2: 4 matmuls ----
            ypre = psA.tile([64, 128], F32, tag="psA", name="ypre")
            for k2y in range(4):
                nc.tensor.matmul(
                    ypre[:],
                    v(W2p, k2y * 64, [[W2p.ap[0][0], 128], [1, 64]]),
                    v(Y2, 18 * k2y,
                      [[Y2.ap[0][0], 128], [324, 2], [36, 8], [2, 8]]),
                    start=(k2y == 0), stop=(k2y == 3))
            yT = wpool.tile([64, 128], F32)
            nc.scalar.activation(yT[:], ypre[:], AF.Relu, bias=b2t[:])
            m2T = wpool.tile([64, 128], F32)
            nc.vector.tensor_scalar(m2T[:], yT[:], 0.0, None, ALU.is_gt)

            # m2 mask in (b,o)-partition layout for phase C
            yTT_ps = psB.tile([128, 64], F32, tag="psB", name="yTT_ps")
            nc.tensor.transpose(yTT_ps[:], yT[:], ident[0:64, 0:64])
            m2g = wpool.tile([128, 64], F32)
            nc.vector.tensor_scalar(m2g[:], yTT_ps[:], 0.0, None, ALU.is_gt)

            def hopfield_f32(src, tag):
                a_ps = psA.tile([128, 512], F32, tag="psA",
                                name=f"a_ps{tag}")
                nc.tensor.matmul(a_ps[:], src, ktK[:], start=True,
                                 stop=True)
                rmax = wpool.tile([128, 1], F32, name=f"rmax{tag}")
                nc.vector.tensor_reduce(rmax[:], a_ps[:], AX.X, ALU.max)
                negbm = wpool.tile([128, 1], F32, name=f"negbm{tag}")
                nc.vector.tensor_scalar(negbm[:], rmax[:], -0.125, None,
                                        ALU.mult)
                p_sb = wpool.tile([128, 512], F32, name=f"p_sb{tag}")
                ssum = wpool.tile([128, 1], F32, name=f"ssum{tag}")
                nc.scalar.activation(p_sb[:], a_ps[:], AF.Exp,
                                     bias=negbm[:], scale=0.125,
                                     accum_out=ssum[:])
                rec = wpool.tile([128, 1], F32, name=f"rec{tag}")
                nc.vector.reciprocal(rec[:], ssum[:])
                nc.vector.tensor_scalar(p_sb[:], p_sb[:], rec[:], None,
                                        ALU.mult)
                yq_ps = psC.tile([128, 64], F32, tag="psC",
                                 name=f"yq_ps{tag}")
                for t in range(4):
                    pt_ps = psB.tile([128, 128], F32, tag="psB",
                                     name=f"pt_ps{tag}{t}")
                    nc.tensor.transpose(pt_ps[:],
                                        p_sb[:, t * 128:(t + 1) * 128],
                                        ident[:])
                    pt_sb = wpool.tile([128, 128], F32, tag="pt_sb",
                                       name=f"pt_sb{tag}{t}")
                    nc.scalar.copy(pt_sb[:], pt_ps[:])
                    nc.tensor.matmul(yq_ps[:], pt_sb[:],
                                     vV[:, t * 64:(t + 1) * 64],
                                     start=(t == 0), stop=(t == 3))
                yq_sb = wpool.tile([128, 64], F32, name=f"yq_sb{tag}")
                nc.scalar.copy(yq_sb[:], yq_ps[:])
                return yq_sb

            yq1 = hopfield_f32(yT[:], "h1")

            yqT_ps = psB.tile([64, 128], F32, tag="psB", name="yqT_ps")
            nc.tensor.transpose(yqT_ps[:], yq1[:], ident[:])
            r2T = wpool.tile([64, 128], F32)
            nc.vector.scalar_tensor_tensor(r2T[:], yqT_ps[:], -1.0, yT[:],
                                           ALU.mult, ALU.add)
            nc.vector.tensor_mul(r2T[:], r2T[:], m2T[:])

            # ---- Phase B: g1 pairs + ep ----
            g1m4 = [wpool.tile([128, 128], F32, name=f"g1m4{g}")
                    for g in range(4)]
            for g in range(4):
                g1ps = psB.tile([128, 128], F32, tag="psB",
                                name=f"g1ps{g}")
                for j in range(2):
                    nc.tensor.matmul(
                        g1ps[64 * j:64 * j + 64, :],
                        v(w2k2m, (2 * g + j) * 64,
                          [[w2k2m.ap[0][0], 64], [1, 64]]),
                        r2T[:], start=True, stop=True)
                for k2l in range(4):
                    nc.vector.tensor_tensor(
                        g1m4[g][k2l * 32:(k2l + 1) * 32, :],
                        g1ps[k2l * 32:(k2l + 1) * 32, :],
                        tapv(m1p4, k2l * 32, 4 * g + k2l, ypitch),
                        ALU.mult)

            ep_ps = psA.tile([100, 128], F32, tag="psA", name="ep_ps")
            for g in range(4):
                nc.tensor.matmul(ep_ps[:], w1stp4[g][:], g1m4[g][:],
                                 start=(g == 0), stop=(g == 3))
            ep_sb = wpool.tile([100, 128], F32)
            nc.scalar.copy(ep_sb[:], ep_ps[:])
            ep2_ps = psB.tile([128, 100], F32, tag="psB", name="ep2_ps")
            nc.tensor.transpose(ep2_ps[:], ep_sb[:], ident[0:100, 0:100])
            ep2 = wpool.tile([128, 100], F32)
            nc.scalar.copy(ep2[:], ep2_ps[:])

            # scatter e-patches to DRAM (single 4-dim DMA)
            for b, dq in ((0, dmaS), (1, dmaA)):
                dq(v(ed4p, b * 14400 + 2 * 1200 + 2 * 100,
                     [[1200, 8], [100, 8], [1, 100]]),
                   ep2[b * 64:(b + 1) * 64, :])

            # data_e (dedup: two distinct 16-row contents + 2 copies)
            de = wpool.tile([128, 3600], F32)
            for g in range(8):
                dq = dmaS if g % 2 == 0 else dmaA
                r = g // 2
                dq(de[g * 16:(g + 1) * 16, :],
                   v(ed4p, 1200 * (1 if r >= 1 else 0),
                     [[14400, 2], [1200, 8], [1, 3600]]))

            e9 = wpool.tile([128, 144], F32)
            e9p = e9.ap[0][0]
            nc.gpsimd.indirect_copy(
                v(e9, 0, [[e9p, 128], [1, 144], [1, 1]]),
                de[:], idxE[:], True)

            # argmin with reference tie semantics (v1 128-row layout)
            mincand = wpool.tile([128, 16], F32)
            nc.vector.tensor_reduce(
                mincand[:], v(e9, 0, [[e9p, 128], [9, 16], [1, 9]]),
                AX.X, ALU.min)
            mstar = wpool.tile([128, 16], F32)
            nc.vector.tensor_scalar(mstar[:], mincand[:], 0.0, None,
                                    ALU.min)
            eq9 = wpool.tile([128, 144], F32)
            nc.vector.tensor_tensor(
                v(eq9, 0, [[eq9.ap[0][0], 128], [9, 16], [1, 9]]),
                v(e9, 0, [[e9p, 128], [9, 16], [1, 9]]),
                v(mstar, 0, [[mstar.ap[0][0], 128], [1, 16], [0, 9]]),
                ALU.is_equal)
            cs = wpool.tile([128, 144], F32)
            nc.vector.scalar_tensor_tensor(cs[:], eq9[:], -1000.0,
                                           oidx9[:], ALU.mult, ALU.add)
            minc2 = wpool.tile([128, 16], F32)
            nc.vector.tensor_reduce(
                minc2[:], v(cs, 0, [[cs.ap[0][0], 128], [9, 16], [1, 9]]),
                AX.X, ALU.min)
            zeq = wpool.tile([128, 16], F32)
            nc.vector.tensor_scalar(zeq[:], mstar[:], 0.0, None,
                                    ALU.is_equal)
            zsc = wpool.tile([128, 16], F32)
            nc.vector.scalar_tensor_tensor(zsc[:], zeq[:], -1000.0,
                                           zc128[:], ALU.mult, ALU.add)
            sel16 = wpool.tile([128, 16], F32)
            nc.vector.tensor_tensor(sel16[:], minc2[:], zsc[:], ALU.min)
            nc.vector.tensor_scalar(sel16[:], sel16[:], 1000.0, None,
                                    ALU.add)

            # permute rows (r,h,b,q)->(b,iy) x ix-halves, ONE 3-dim scatter
            selbi_ps = psC.tile([64, 32], F32, tag="psC", name="selbi_ps")
            nc.tensor.matmul(selbi_ps[:, 0:16], permA[:], sel16[:],
                             start=True, stop=True)
            nc.tensor.matmul(selbi_ps[:, 16:32], permB[:], sel16[:],
                             start=True, stop=True)
            selbi = wpool.tile([64, 32], BF16)
            nc.scalar.copy(selbi[:], selbi_ps[:])
            dmaS(v(sel_pad, 117, [[1444, 2], [38, 32], [1, 32]]),
                 v(selbi, 0, [[selbi.ap[0][0], 64], [1, 32]]))

            # sel image rows with (k1y,k1x) shifts, replicated x8
            ds = wpool.tile([128, 2888], BF16)
            qs = (dmaS, dmaA, dmaS, dmaA, dmaS, dmaA, dmaS, dmaA)
            for k in range(8):
                qs[k](ds[k * 16:(k + 1) * 16, :],
                      v(sel_pad, 0, [[38, 4], [1, 4], [1, 2888]]))

            selm2 = []
            for s in range(2):
                sg = wpool.tile([128, 128], BF16, name=f"sg{s}")
                nc.gpsimd.indirect_copy(
                    v(sg, 0, [[sg.ap[0][0], 128], [1, 128], [1, 1]]),
                    ds[:], ixS[:, s * 8:(s + 1) * 8], True)
                sgb = wpool.tile([128, 128], BF16, name=f"sgb{s}")
                nc.vector.tensor_tensor(sgb[:], sg[:], oidx128[:],
                                        ALU.is_equal)
                selm2.append(sgb)

            # ---- Phase C (bf16): masks -> xsel -> z pairs -> ym ----
            xsel = []
            for t in range(8):
                mx_ps = psB.tile([128, 128], F32, tag="psB",
                                 name=f"mx{t}")
                nc.tensor.matmul(mx_ps[:],
                                 emt8[:, t * 128:(t + 1) * 128],
                                 selm2[(t // 2) // 2][:],
                                 start=True, stop=True)
                xs = wpool.tile([128, 128], BF16, name=f"xs{t}")
                nc.vector.tensor_tensor(xs[:], xg[t][:], mx_ps[:],
                                        ALU.mult)
                xsel.append(xs)

            zm4 = [wpool.tile([128, 128], BF16, name=f"zm4{g}")
                   for g in range(4)]
            for g in range(4):
                zps = psB.tile([128, 128], F32, tag="psB", name=f"zps{g}")
                for k2xh in range(2):
                    t = g * 2 + k2xh
                    nc.tensor.matmul(zps[64 * k2xh:64 * k2xh + 64, :],
                                     w1fpbd[:], xsel[t][:],
                                     start=True, stop=True)
                for k2x in range(4):
                    nc.vector.tensor_tensor(
                        zm4[g][k2x * 32:(k2x + 1) * 32, :],
                        zps[k2x * 32:(k2x + 1) * 32, :],
                        tapv(m1p4, k2x * 32, 4 * g + k2x, ypitch),
                        ALU.mult)

            ym_ps = psC.tile([128, 64], F32, tag="psC", name="ym_ps")
            for g in range(4):
                nc.tensor.matmul(ym_ps[:], zm4[g][:],
                                 w2pk[:, g * 64:(g + 1) * 64],
                                 start=(g == 0), stop=(g == 3))
            ymm = wpool.tile([128, 64], BF16)
            nc.vector.tensor_tensor(ymm[:], ym_ps[:], m2g[:], ALU.mult)

            t2_ps = psB.tile([64, 128], BF16, tag="psB", name="t2_ps")
            nc.tensor.transpose(t2_ps[:], ymm[:], identb[:])
            ymmT = wpool.tile([64, 128], BF16)
            nc.scalar.copy(ymmT[:], t2_ps[:])

            # hopfield 2 in bf16
            a_ps = psA.tile([128, 512], F32, tag="psA", name="a_ps2")
            nc.tensor.matmul(a_ps[:], ymmT[:], ktb[:], start=True,
                             stop=True)
            rmax = wpool.tile([128, 1], F32, name="rmax2")
            nc.vector.tensor_reduce(rmax[:], a_ps[:], AX.X, ALU.max)
            negbm = wpool.tile([128, 1], F32, name="negbm2")
            nc.vector.tensor_scalar(negbm[:], rmax[:], -0.125, None,
                                    ALU.mult)
            p_sb = wpool.tile([128, 512], BF16, name="p_sb2")
            ssum = wpool.tile([128, 1], F32, name="ssum2")
            nc.scalar.activation(p_sb[:], a_ps[:], AF.Exp, bias=negbm[:],
                                 scale=0.125, accum_out=ssum[:])
            rec = wpool.tile([128, 1], F32, name="rec2")
            nc.vector.reciprocal(rec[:], ssum[:])
            nc.vector.tensor_scalar(p_sb[:], p_sb[:], rec[:], None,
                                    ALU.mult)
            yq2_ps = psC.tile([128, 64], F32, tag="psC", name="yq2_ps")
            for t in range(4):
                pt_ps = psB.tile([128, 128], BF16, tag="psB",
                                 name=f"pt2_{t}")
                nc.tensor.transpose(pt_ps[:],
                                    p_sb[:, t * 128:(t + 1) * 128],
                                    identb[:])
                pt_sb = wpool.tile([128, 128], BF16, tag="pt_sb2",
                                   name=f"pt_sb2{t}")
                nc.scalar.copy(pt_sb[:], pt_ps[:])
                nc.tensor.matmul(yq2_ps[:], pt_sb[:],
                                 vb[:, t * 64:(t + 1) * 64],
                                 start=(t == 0), stop=(t == 3))
            yq2 = wpool.tile([128, 64], F32)
            nc.scalar.copy(yq2[:], yq2_ps[:])

            tr_ps = psB.tile([64, 128], F32, tag="psB", name="tr_ps")
            nc.tensor.transpose(tr_ps[:], yq2[:], ident[:])
            outT = wpool.tile([64, 128], F32)
            nc.scalar.copy(outT[:], tr_ps[:])
            dmaS(AP(out_d, 0, [[64, 64], [4096, 2], [8, 8], [1, 8]]),
                 v(outT, 0, [[outT.ap[0][0], 64], [1, 128]]))

    return nc


_CACHE = {}


def kernel(**inputs) -> np.ndarray:
    from concourse.bass_utils import run_bass_kernel_spmd
    if "nc" not in _CACHE:
        from concourse import bacc
        nc = bacc.Bacc("TRN2", target_bir_lowering=False, debug=False,
                       num_devices=N_CORES)
        build_program(nc)
        nc.compile()
        _CACHE["nc"] = nc
    nc = _CACHE["nc"]
    feed = prepare_feed(inputs)
    in_maps = [dict(feed) for _ in range(N_CORES)]
    res = run_bass_kernel_spmd(nc, in_maps, list(range(N_CORES)))
    return np.asarray(res.results[0]["out"], np.float32)


# revision 12
# speedup vs baseline: 1.9545x; 1.0556x over previous
"""Trainium2 Bass kernel for nn_Block1_87144886436577 (vq_codebook), v2.

Analytic collapse of the reference's jacobians (see v1 docstring), with:
- conv1 as ONE fp32 matmul (im2col rows (ci,k1y,k1x) built by 3 DMAs
  from the host-padded image, replicated 4x in partition blocks),
- conv2 as 4 fp32 matmuls contracting (k2x, m)=128 per k2y over a
  shifted-row copy of y1,
- phase-B backprop (g1/ep) packed into 8+4 fp32 matmuls via
  host-rearranged weights; fp32 is REQUIRED upstream of the argmin
  (sel margins are ~1e-3; bf16 flips selections),
- the e-patch/sel DRAM round-trips deduplicated + spread across the
  sync/scalar/gpsimd DMA queues, argmin on a 64-partition layout, a
  permutation matmul so sel scatters with a single 3-dim DMA,
- everything downstream of sel (mask expansion, masked conv taps,
  second hopfield) in bf16 with tap-pair packing,
- all weight repacks/transposes done host-side (pure layout + casts).

All 8 cores run identical replicas; output read from core 0.
"""
import sys

import numpy as np

for _p in ("/opt/trn_rl_repo",):
    if _p not in sys.path:
        sys.path.insert(0, _p)

import concourse.bass as bass
import concourse.mybir as mybir
import concourse.tile as tile

F32 = mybir.dt.float32
BF16 = mybir.dt.bfloat16
U16 = mybir.dt.uint16
AF = mybir.ActivationFunctionType
ALU = mybir.AluOpType
AX = mybir.AxisListType
AP = bass.AP

N_CORES = 8


def v(t, off, pat):
    """Custom-view AP over a tile (t = AP returned by pool.tile)."""
    return AP(t.tensor, t.offset + off, pat)


def _e(r):
    return 1 if r >= 1 else 0


def _tables():
    """Input-independent index/mask tables."""
    import ml_dtypes
    BF = ml_dtypes.bfloat16

    ident = np.eye(128, dtype=np.float32)
    identb = ident.astype(BF)
    oidx128 = np.tile((np.arange(128) % 64).astype(BF)[None, :],
                      (128, 1))

    # xsel gather streams (same as v1): tile t=(k2y,k2xh); partition
    # p=k2xp*64+k1y*16+k1x*4+ci; j<128 per tile: (b,oy,ox).
    idxX = np.zeros((8, 128, 8), np.uint16)
    for t in range(8):
        k2y, k2xh = t // 2, t % 2
        for g in range(8):
            k2xp = g // 4
            k1y = g % 4
            k2x = 2 * k2xh + k2xp
            for j in range(128):
                b, oy, ox = j // 64, (j % 64) // 8, j % 8
                idxX[t, 16 * g + j % 16, j // 16] = (
                    b * 1444 + (4 * oy + 2 * k2y + k1y) * 38
                    + 4 * ox + 2 * k2x)
    ixX = np.zeros((128, 64), np.uint16)
    for t in range(8):
        ixX[:, t * 8:(t + 1) * 8] = idxX[t]

    # sel gather streams (as v1)
    idxS = np.zeros((2, 128, 8), np.uint16)
    for s in range(2):
        for g in range(8):
            k2yp, k2x = g // 4, g % 4
            k2y = s * 2 + k2yp
            for j in range(128):
                b, oy, ox = j // 64, (j % 64) // 8, j % 8
                idxS[s, 16 * g + j % 16, j // 16] = (
                    b * 1444 + (4 * oy + 2 * k2y) * 38 + 4 * ox + 2 * k2x)
    ixS = np.zeros((128, 16), np.uint16)
    for s in range(2):
        ixS[:, s * 8:(s + 1) * 8] = idxS[s]

    # mask expansion matrices, bf16: emt8[r, t*128+p]
    emt8 = np.zeros((128, 1024), np.float32)
    for t in range(8):
        k2y, k2xh = t // 2, t % 2
        for p in range(128):
            k2xp, k1y, k1x = p // 64, (p % 64) // 16, p % 4
            k2x = 2 * k2xh + k2xp
            r = (k2y % 2) * 64 + k2x * 16 + k1y * 4 + k1x
            emt8[r, t * 128 + p] = 1.0
    emt8 = emt8.astype(BF)

    # E9 gather (v1 layout): p = r*32+h*16+b*8+q (iy=4q+r, ix=16h+ixl),
    # stream j = ixl*9 + jj. data row = ed4p[b, q+e(r) : +3 rows] flat.
    idxE = np.zeros((128, 9), np.uint16)
    oidx9 = np.full((128, 144), 3000.0, np.float32)
    zc128 = np.zeros((128, 16), np.float32)
    for r in range(4):
        for h in range(2):
            g = r * 2 + h
            for j in range(144):
                ixl, jj = j // 9, j % 9
                jy, jx = jj // 3, jj % 3
                t_ = ixl % 4
                s = 4 * h + ixl // 4
                dy = r - 4 * _e(r) + 4 * jy + 3
                dx = t_ - 4 * _e(t_) + 4 * jx + 3
                ox = s + _e(t_) - jx
                if 0 <= dy < 10 and 0 <= dx < 10:
                    idx = jy * 120 + (ox + 2) * 10 + dx
                else:
                    idx = 0  # guaranteed-zero pad cell (oxp=0 col)
                idxE[16 * g + j % 16, j // 16] = idx
    for r in range(4):
        for h in range(2):
            for b in range(2):
                for q in range(8):
                    p = r * 32 + h * 16 + b * 8 + q
                    iy = 4 * q + r
                    for ixl in range(16):
                        ix = 16 * h + ixl
                        t_ = ix % 4
                        s = ix // 4
                        for jj in range(9):
                            jy, jx = jj // 3, jj % 3
                            oy = q + _e(r) - jy
                            ox = s + _e(t_) - jx
                            dy = iy - 4 * oy + 3
                            dx = ix - 4 * ox + 3
                            if (0 <= oy < 8 and 0 <= ox < 8
                                    and 0 <= dy < 10 and 0 <= dx < 10):
                                oidx9[p, ixl * 9 + jj] = oy * 8 + ox
                        for o in range(64):
                            oy, ox = o // 8, o % 8
                            if not (0 <= iy - 4 * oy + 3 < 10
                                    and 0 <= ix - 4 * ox + 3 < 10):
                                zc128[p, ixl] = float(o)
                                break

    # permutations: sel16 rows (r,h,b,q) -> rows (b, iy), cols ix-halves
    permA = np.zeros((128, 64), np.float32)
    permB = np.zeros((128, 64), np.float32)
    for r in range(4):
        for h in range(2):
            for b in range(2):
                for q in range(8):
                    p = r * 32 + h * 16 + b * 8 + q
                    (permA if h == 0 else permB)[p, b * 32 + 4 * q + r] \
                        = 1.0

    neg1s = np.full((2, 1520), -1.0, BF)
    zed = np.zeros((12, 2400), np.float32)
    return {"ident": ident, "identb": identb, "oidx128": oidx128,
            "ixX": ixX, "ixS": ixS, "emt8": emt8, "idxE": idxE,
            "oidx9": oidx9, "zc128": zc128, "permA": permA,
            "permB": permB, "neg1s": neg1s, "zed": zed}


def prepare_feed(inputs):
    """Host-side layout/cast-only rearrangements of the inputs."""
    import ml_dtypes
    BF = ml_dtypes.bfloat16
    x = np.asarray(inputs["x"], np.float32)    # (2,3,32,32)
    w1 = np.asarray(inputs["w1"], np.float32)  # (32,3,4,4)
    b1 = np.asarray(inputs["b1"], np.float32)  # (32,)
    w2 = np.asarray(inputs["w2"], np.float32)  # (64,32,4,4)
    b2 = np.asarray(inputs["b2"], np.float32)  # (64,)
    K = np.asarray(inputs["K"], np.float32)    # (512,64)
    V = np.asarray(inputs["V"], np.float32)    # (512,64)

    f = dict(_TABLES)

    xpad = np.zeros((3, 3008), np.float32)
    img = np.zeros((3, 2, 38, 38), np.float32)
    img[:, :, 3:35, 3:35] = x.transpose(1, 0, 2, 3)
    xpad[:, 0:2888] = img.reshape(3, 2888)
    f["xpad3"] = xpad

    # data_x rows (ci,k1x): padded image shifted left by k1x; ci=3 zero.
    dx16 = np.zeros((16, 2888), np.float32)
    for ci in range(3):
        for k1x in range(4):
            dx16[ci * 4 + k1x, 0:2888 - k1x] = \
                f["xpad3"][ci, k1x:2888]
    f["dx16"] = dx16.astype(BF)

    # conv1 weights: [ci*16+k1y*4+k1x, j*32+m], 4 dup col blocks
    w1t = w1.transpose(1, 2, 3, 0).reshape(48, 32)
    f["w1f4"] = np.tile(w1t, (1, 4)).copy()
    f["b1t4"] = np.tile(b1, 4).reshape(128, 1).copy()

    # conv2 weights: [k2x*32+m, k2y*64+c]
    W2p = np.zeros((128, 256), np.float32)
    for k2y in range(4):
        for k2x in range(4):
            W2p[k2x * 32:(k2x + 1) * 32, k2y * 64:(k2y + 1) * 64] = \
                w2[:, :, k2y, k2x].T
    f["W2p"] = W2p
    f["b2t"] = b2.reshape(64, 1).copy()

    f["ktK"] = K.T.copy()                       # [64, 512]
    f["vV"] = V.reshape(4, 128, 64).transpose(1, 0, 2).reshape(128, 256) \
        .copy()                                 # [128, (t,c)]
    f["ktb"] = f["ktK"].astype(BF)
    f["vb"] = f["vV"].astype(BF)

    # g1 pair weights: [c, k2*32+m]
    w2k2m = np.zeros((64, 512), np.float32)
    for k2 in range(16):
        k2y, k2x = k2 // 4, k2 % 4
        w2k2m[:, k2 * 32:(k2 + 1) * 32] = w2[:, :, k2y, k2x]
    f["w2k2m"] = w2k2m

    # w1 summed-tap images, built on chip from w1sb4
    f["w1sb4"] = np.tile(w1.reshape(32, 48), (4, 1)).copy()

    # z-pair blockdiag weights bf16 [128, 64]
    w1fp64 = np.zeros((64, 32), np.float32)
    for k1y in range(4):
        for ci in range(3):
            for k1x in range(4):
                w1fp64[k1y * 16 + ci * 4 + k1x, :] = w1[:, ci, k1y, k1x]
    bd = np.zeros((128, 64), np.float32)
    bd[0:64, 0:32] = w1fp64
    bd[64:128, 32:64] = w1fp64
    f["w1fpbd"] = bd.astype(BF)

    # ym pack weights bf16 [128, (g,c)]
    w2pk = np.zeros((128, 256), np.float32)
    for g in range(4):
        for k2x in range(4):
            w2pk[k2x * 32:(k2x + 1) * 32, g * 64:(g + 1) * 64] = \
                w2[:, :, g, k2x].T
    f["w2pk"] = w2pk.astype(BF)
    return f


_TABLES = _tables()


def build_program(nc):
    def P(name, shape, dt):
        return nc.declare_dram_parameter(name, shape, dt, isOutput=False)

    xpad_d = P("xpad3", [3, 3008], F32)
    dx16_d = P("dx16", [16, 2888], BF16)
    w1f4_d = P("w1f4", [48, 128], F32)
    b1t4_d = P("b1t4", [128, 1], F32)
    W2p_d = P("W2p", [128, 256], F32)
    b2t_d = P("b2t", [64, 1], F32)
    ktK_d = P("ktK", [64, 512], F32)
    vV_d = P("vV", [128, 256], F32)
    ktb_d = P("ktb", [64, 512], BF16)
    vb_d = P("vb", [128, 256], BF16)
    w2k2m_d = P("w2k2m", [64, 512], F32)
    w1sb4_d = P("w1sb4", [128, 48], F32)
    w1fpbd_d = P("w1fpbd", [128, 64], BF16)
    w2pk_d = P("w2pk", [128, 256], BF16)
    ident_d = P("ident", [128, 128], F32)
    identb_d = P("identb", [128, 128], BF16)
    oidx128_d = P("oidx128", [128, 128], BF16)
    ixX_d = P("ixX", [128, 64], U16)
    ixS_d = P("ixS", [128, 16], U16)
    emt8_d = P("emt8", [128, 1024], BF16)
    idxE_d = P("idxE", [128, 9], U16)
    oidx9_d = P("oidx9", [128, 144], F32)
    zc128_d = P("zc128", [128, 16], F32)
    permA_d = P("permA", [128, 64], F32)
    permB_d = P("permB", [128, 64], F32)
    neg1s_d = P("neg1s", [2, 1520], BF16)
    zed_d = P("zed", [12, 2400], F32)
    out_d = nc.declare_dram_parameter("out", [2, 64, 8, 8], F32,
                                      isOutput=True)

    with tile.TileContext(nc) as tc:
        with (
            tc.tile_pool(name="const", bufs=1) as cpool,
            tc.tile_pool(name="work", bufs=1) as wpool,
            tc.tile_pool(name="psA", bufs=2, space="PSUM") as psA,
            tc.tile_pool(name="psB", bufs=4, space="PSUM") as psB,
            tc.tile_pool(name="psC", bufs=2, space="PSUM") as psC,
            tc.tile_pool(name="dram", bufs=1, space="DRAM") as dpool,
        ):
            dmaS = nc.sync.dma_start
            dmaA = nc.scalar.dma_start
            dmaG = nc.gpsimd.dma_start

            # ---- DRAM scratch ----
            ed4p = dpool.tile([2, 12, 12, 100], F32)
            edT = dpool.tile([24, 1200], F32)
            sel_pad = dpool.tile([3040], BF16)

            # ---- critical-path loads first (sync queue) ----
            w1f4 = wpool.tile([48, 128], F32)
            dmaS(w1f4[:], w1f4_d[:])
            X48 = wpool.tile([48, 2888], F32)
            for ci, dq in ((0, dmaS), (1, dmaA), (2, dmaS)):
                dq(X48[ci * 16:(ci + 1) * 16, :],
                   AP(xpad_d, ci * 3008, [[38, 4], [1, 4], [1, 2888]]))
            b1t4 = wpool.tile([128, 1], F32)
            dmaS(b1t4[:], b1t4_d[:])
            W2p = wpool.tile([128, 256], F32)
            dmaS(W2p[:], W2p_d[:])
            b2t = wpool.tile([64, 1], F32)
            dmaS(b2t[:], b2t_d[:])
            ktK = wpool.tile([64, 512], F32)
            dmaS(ktK[:], ktK_d[:])
            ident = cpool.tile([128, 128], F32)
            dmaS(ident[:], ident_d[:])
            vV = wpool.tile([128, 256], F32)
            dmaS(vV[:], vV_d[:])
            w2k2m = wpool.tile([64, 512], F32)
            dmaA(w2k2m[:], w2k2m_d[:])
            w1sb4 = wpool.tile([128, 48], F32)
            dmaA(w1sb4[:], w1sb4_d[:])

            # ---- non-critical loads (scalar/gpsimd queues) ----
            dmaA(v(ed4p, 0, [[2400, 12], [1, 2400]]), zed_d[:])
            dmaA(v(sel_pad, 0, [[1520, 2], [1, 1520]]), neg1s_d[:])
            ixX = cpool.tile([128, 64], U16)
            dmaS(ixX[:], ixX_d[:])
            dx = wpool.tile([128, 2888], BF16)
            dmaS(dx[0:16, :], dx16_d[:])
            for d in (16, 32, 64):
                dmaS(dx[d:2 * d, :], dx[0:d, :])
            identb = cpool.tile([128, 128], BF16)
            dmaA(identb[:], identb_d[:])
            oidx128 = cpool.tile([128, 128], BF16)
            dmaA(oidx128[:], oidx128_d[:])
            ixS = cpool.tile([128, 16], U16)
            dmaA(ixS[:], ixS_d[:])
            emt8 = cpool.tile([128, 1024], BF16)
            dmaA(emt8[:], emt8_d[:])
            idxE = cpool.tile([128, 9], U16)
            dmaA(idxE[:], idxE_d[:])
            oidx9 = cpool.tile([128, 144], F32)
            dmaA(oidx9[:], oidx9_d[:])
            zc128 = cpool.tile([128, 16], F32)
            dmaA(zc128[:], zc128_d[:])
            permA = cpool.tile([128, 64], F32)
            dmaA(permA[:], permA_d[:])
            permB = cpool.tile([128, 64], F32)
            dmaA(permB[:], permB_d[:])
            ktb = wpool.tile([64, 512], BF16)
            dmaA(ktb[:], ktb_d[:])
            vb = wpool.tile([128, 256], BF16)
            dmaA(vb[:], vb_d[:])
            w1fpbd = wpool.tile([128, 64], BF16)
            dmaA(w1fpbd[:], w1fpbd_d[:])
            w2pk = wpool.tile([128, 256], BF16)
            dmaA(w2pk[:], w2pk_d[:])

            # x-side gathers (overlap phases A/B)
            xg = [wpool.tile([128, 128], BF16, name=f"xg{t}")
                  for t in range(8)]
            for t in range(8):
                nc.gpsimd.indirect_copy(
                    v(xg[t], 0, [[xg[t].ap[0][0], 128], [1, 128], [1, 1]]),
                    dx[:], ixX[:, t * 8:(t + 1) * 8], True)

            # w1 summed-tap images -> w1stp4[g] [128(k2x,m), 100]
            w1s4 = wpool.tile([128, 16], F32)
            nc.vector.tensor_reduce(
                w1s4[:],
                v(w1sb4, 0, [[w1sb4.ap[0][0], 128], [1, 16], [16, 3]]),
                AX.X, ALU.add)
            w1si = wpool.tile([128, 256], F32)
            nc.vector.memset(w1si[:], 0.0)
            nc.vector.tensor_copy(
                v(w1si, 102, [[w1si.ap[0][0], 128], [16, 4], [1, 4]]),
                w1s4[:])
            w1stp4 = [wpool.tile([128, 100], F32, name=f"w1stp4{g}")
                      for g in range(4)]
            wpitch = w1si.ap[0][0]
            for g in range(4):
                for k2x in range(4):
                    dst = w1stp4[g]
                    nc.vector.tensor_copy(
                        v(dst, k2x * 32 * dst.ap[0][0],
                          [[dst.ap[0][0], 32], [10, 10], [1, 10]]),
                        v(w1si, k2x * 32 * wpitch
                          + (6 - 2 * g) * 16 + 6 - 2 * k2x,
                          [[wpitch, 32], [16, 10], [1, 10]]))

            # ---- Phase A: conv1 (one matmul) ----
            y1ps = psA.tile([128, 512], F32, tag="psA", name="y1ps")
            nc.tensor.matmul(
                y1ps[:], w1f4[:],
                v(X48, 78, [[X48.ap[0][0], 48], [1444, 2], [76, 16],
                            [2, 16]]),
                start=True, stop=True)
            y1sb = wpool.tile([128, 512], F32)
            nc.scalar.activation(y1sb[:], y1ps[:], AF.Relu, bias=b1t4[:])

            y1p4 = wpool.tile([128, 648], F32)
            nc.vector.memset(y1p4[:], 0.0)
            ypitch = y1p4.ap[0][0]
            nc.vector.tensor_copy(
                v(y1p4, 19, [[ypitch, 128], [324, 2], [18, 16], [1, 16]]),
                v(y1sb, 0,
                  [[y1sb.ap[0][0], 128], [256, 2], [16, 16], [1, 16]]))
            m1p4 = wpool.tile([128, 648], F32)
            nc.vector.tensor_scalar(m1p4[:], y1p4[:], 0.0, None, ALU.is_gt)

            # shifted-row copy for conv2 im2col (block k2x shifted by k2x)
            Y2 = wpool.tile([128, 648], F32)
            nc.vector.tensor_copy(Y2[0:32, :], y1p4[0:32, :])
            nc.vector.tensor_copy(Y2[32:64, 0:647], y1p4[32:64, 1:648])
            nc.scalar.copy(Y2[64:96, 0:646], y1p4[64:96, 2:648])
            nc.scalar.copy(Y2[96:128, 0:645], y1p4[96:128, 3:648])

            def tapv(tl, pbase, k2, pitch, n=32):
                k2y, k2x = k2 // 4, k2 % 4
                return v(tl, pbase * pitch + 18 * k2y + k2x,
                         [[pitch, n], [324, 2], [36, 8], [2, 8]])

            # ---- conv2: 4 matmuls ----
            ypre = psA.tile([64, 128], F32, tag="psA", name="ypre")
            for k2y in range(4):
                nc.tensor.matmul(
                    ypre[:],
                    v(W2p, k2y * 64, [[W2p.ap[0][0], 128], [1, 64]]),
                    v(Y2, 18 * k2y,
                      [[Y2.ap[0][0], 128], [324, 2], [36, 8], [2, 8]]),
                    start=(k2y == 0), stop=(k2y == 3))
            yT = wpool.tile([64, 128], F32)
            nc.scalar.activation(yT[:], ypre[:], AF.Relu, bias=b2t[:])
            m2T = wpool.tile([64, 128], F32)
            nc.vector.tensor_scalar(m2T[:], yT[:], 0.0, None, ALU.is_gt)

            # m2 mask in (b,o)-partition layout for phase C
            yTT_ps = psB.tile([128, 64], F32, tag="psB", name="yTT_ps")
            nc.tensor.transpose(yTT_ps[:], yT[:], ident[0:64, 0:64])
            m2g = wpool.tile([128, 64], F32)
            nc.vector.tensor_scalar(m2g[:], yTT_ps[:], 0.0, None, ALU.is_gt)

            def hopfield_f32(src, tag):
                a_ps = psA.tile([128, 512], F32, tag="psA",
                                name=f"a_ps{tag}")
                nc.tensor.matmul(a_ps[:], src, ktK[:], start=True,
                                 stop=True)
                rmax = wpool.tile([128, 1], F32, name=f"rmax{tag}")
                nc.vector.tensor_reduce(rmax[:], a_ps[:], AX.X, ALU.max)
                negbm = wpool.tile([128, 1], F32, name=f"negbm{tag}")
                nc.vector.tensor_scalar(negbm[:], rmax[:], -0.125, None,
                                        ALU.mult)
                p_sb = wpool.tile([128, 512], F32, name=f"p_sb{tag}")
                ssum = wpool.tile([128, 1], F32, name=f"ssum{tag}")
                nc.scalar.activation(p_sb[:], a_ps[:], AF.Exp,
                                     bias=negbm[:], scale=0.125,
                                     accum_out=ssum[:])
                rec = wpool.tile([128, 1], F32, name=f"rec{tag}")
                nc.vector.reciprocal(rec[:], ssum[:])
                nc.vector.tensor_scalar(p_sb[:], p_sb[:], rec[:], None,
                                        ALU.mult)
                yq_ps = psC.tile([128, 64], F32, tag="psC",
                                 name=f"yq_ps{tag}")
                for t in range(4):
                    pt_ps = psB.tile([128, 128], F32, tag="psB",
                                     name=f"pt_ps{tag}{t}")
                    nc.tensor.transpose(pt_ps[:],
                                        p_sb[:, t * 128:(t + 1) * 128],
                                        ident[:])
                    pt_sb = wpool.tile([128, 128], F32, tag="pt_sb",
                                       name=f"pt_sb{tag}{t}")
                    nc.scalar.copy(pt_sb[:], pt_ps[:])
                    nc.tensor.matmul(yq_ps[:], pt_sb[:],
                                     vV[:, t * 64:(t + 1) * 64],
                                     start=(t == 0), stop=(t == 3))
                yq_sb = wpool.tile([128, 64], F32, name=f"yq_sb{tag}")
                nc.scalar.copy(yq_sb[:], yq_ps[:])
                return yq_sb

            yq1 = hopfield_f32(yT[:], "h1")

            yqT_ps = psB.tile([64, 128], F32, tag="psB", name="yqT_ps")
            nc.tensor.transpose(yqT_ps[:], yq1[:], ident[:])
            r2T = wpool.tile([64, 128], F32)
            nc.vector.scalar_tensor_tensor(r2T[:], yqT_ps[:], -1.0, yT[:],
                                           ALU.mult, ALU.add)
            nc.vector.tensor_mul(r2T[:], r2T[:], m2T[:])

            # ---- Phase B: g1 pairs + ep ----
            g1m4 = [wpool.tile([128, 128], F32, name=f"g1m4{g}")
                    for g in range(4)]
            for g in range(4):
                g1ps = psB.tile([128, 128], F32, tag="psB",
                                name=f"g1ps{g}")
                for j in range(2):
                    nc.tensor.matmul(
                        g1ps[64 * j:64 * j + 64, :],
                        v(w2k2m, (2 * g + j) * 64,
                          [[w2k2m.ap[0][0], 64], [1, 64]]),
                        r2T[:], start=True, stop=True)
                for k2l in range(4):
                    nc.vector.tensor_tensor(
                        g1m4[g][k2l * 32:(k2l + 1) * 32, :],
                        g1ps[k2l * 32:(k2l + 1) * 32, :],
                        tapv(m1p4, k2l * 32, 4 * g + k2l, ypitch),
                        ALU.mult)

            ep_ps = psA.tile([100, 128], F32, tag="psA", name="ep_ps")
            for g in range(4):
                nc.tensor.matmul(ep_ps[:], w1stp4[g][:], g1m4[g][:],
                                 start=(g == 0), stop=(g == 3))
            ep_sb = wpool.tile([100, 128], F32)
            nc.scalar.copy(ep_sb[:], ep_ps[:])
            ep2_ps = psB.tile([128, 100], F32, tag="psB", name="ep2_ps")
            nc.tensor.transpose(ep2_ps[:], ep_sb[:], ident[0:100, 0:100])
            ep2 = wpool.tile([128, 100], F32)
            nc.scalar.copy(ep2[:], ep2_ps[:])

            # scatter e-patches to DRAM (single 4-dim DMA)
            for b, dq in ((0, dmaS), (1, dmaA)):
                dq(v(ed4p, b * 14400 + 2 * 1200 + 2 * 100,
                     [[1200, 8], [100, 8], [1, 100]]),
                   ep2[b * 64:(b + 1) * 64, :])

            # data_e (dedup: two distinct 16-row contents + 2 copies)
            # compact e-patch windows: one clean image load, DVE
            # reorder (oxp between dy and dx), write back, 16 tiny loads
            edS = wpool.tile([24, 1200], F32)
            dmaS(edS[:], v(ed4p, 0, [[1200, 24], [1, 1200]]))
            edR = wpool.tile([24, 1200], F32)
            nc.vector.tensor_copy(
                v(edR, 0, [[edR.ap[0][0], 24], [1, 1200]]),
                v(edS, 0, [[edS.ap[0][0], 24], [10, 10], [100, 12],
                           [1, 10]]))
            dmaS(v(edT, 0, [[1200, 24], [1, 1200]]),
                 edR[0:24, :])
            de = wpool.tile([128, 360], F32)
            for g in range(8):
                r, h = g // 2, g % 2
                e = 1 if r >= 1 else 0
                for b in range(2):
                    dq = dmaS if (g + b) % 2 == 0 else dmaA
                    dq(de[g * 16 + b * 8:g * 16 + b * 8 + 8, :],
                       v(edT, (b * 12 + e + 2) * 1200
                         + (r - 4 * e + 3) * 120,
                         [[1200, 8], [-720, 3], [1, 120]]))

            e9 = wpool.tile([128, 144], F32)
            e9p = e9.ap[0][0]
            nc.gpsimd.indirect_copy(
                v(e9, 0, [[e9p, 128], [1, 144], [1, 1]]),
                de[:], idxE[:], True)

            # argmin with reference tie semantics (v1 128-row layout)
            mincand = wpool.tile([128, 16], F32)
            nc.vector.tensor_reduce(
                mincand[:], v(e9, 0, [[e9p, 128], [9, 16], [1, 9]]),
                AX.X, ALU.min)
            mstar = wpool.tile([128, 16], F32)
            nc.vector.tensor_scalar(mstar[:], mincand[:], 0.0, None,
                                    ALU.min)
            eq9 = wpool.tile([128, 144], F32)
            nc.vector.tensor_tensor(
                v(eq9, 0, [[eq9.ap[0][0], 128], [9, 16], [1, 9]]),
                v(e9, 0, [[e9p, 128], [9, 16], [1, 9]]),
                v(mstar, 0, [[mstar.ap[0][0], 128], [1, 16], [0, 9]]),
                ALU.is_equal)
            cs = wpool.tile([128, 144], F32)
            nc.vector.scalar_tensor_tensor(cs[:], eq9[:], -1000.0,
                                           oidx9[:], ALU.mult, ALU.add)
            minc2 = wpool.tile([128, 16], F32)
            nc.vector.tensor_reduce(
                minc2[:], v(cs, 0, [[cs.ap[0][0], 128], [9, 16], [1, 9]]),
                AX.X, ALU.min)
            zeq = wpool.tile([128, 16], F32)
            nc.vector.tensor_scalar(zeq[:], mstar[:], 0.0, None,
                                    ALU.is_equal)
            zsc = wpool.tile([128, 16], F32)
            nc.vector.scalar_tensor_tensor(zsc[:], zeq[:], -1000.0,
                                           zc128[:], ALU.mult, ALU.add)
            sel16 = wpool.tile([128, 16], F32)
            nc.vector.tensor_tensor(sel16[:], minc2[:], zsc[:], ALU.min)
            nc.vector.tensor_scalar(sel16[:], sel16[:], 1000.0, None,
                                    ALU.add)

            # permute rows (r,h,b,q)->(b,iy) x ix-halves, ONE 3-dim scatter
            selbi_ps = psC.tile([64, 32], F32, tag="psC", name="selbi_ps")
            nc.tensor.matmul(selbi_ps[:, 0:16], permA[:], sel16[:],
                             start=True, stop=True)
            nc.tensor.matmul(selbi_ps[:, 16:32], permB[:], sel16[:],
                             start=True, stop=True)
            selbi = wpool.tile([64, 32], BF16)
            nc.scalar.copy(selbi[:], selbi_ps[:])
            dmaS(v(sel_pad, 117, [[1444, 2], [38, 32], [1, 32]]),
                 v(selbi, 0, [[selbi.ap[0][0], 64], [1, 32]]))

            # sel image rows with (k1y,k1x) shifts, replicated x8
            ds = wpool.tile([128, 2888], BF16)
            qs = (dmaS, dmaA, dmaS, dmaA, dmaS, dmaA, dmaS, dmaA)
            for k in range(8):
                qs[k](ds[k * 16:(k + 1) * 16, :],
                      v(sel_pad, 0, [[38, 4], [1, 4], [1, 2888]]))

            selm2 = []
            for s in range(2):
                sg = wpool.tile([128, 128], BF16, name=f"sg{s}")
                nc.gpsimd.indirect_copy(
                    v(sg, 0, [[sg.ap[0][0], 128], [1, 128], [1, 1]]),
                    ds[:], ixS[:, s * 8:(s + 1) * 8], True)
                sgb = wpool.tile([128, 128], BF16, name=f"sgb{s}")
                nc.vector.tensor_tensor(sgb[:], sg[:], oidx128[:],
                                        ALU.is_equal)
                selm2.append(sgb)

            # ---- Phase C (bf16): masks -> xsel -> z pairs -> ym ----
            xsel = []
            for t in range(8):
                mx_ps = psB.tile([128, 128], F32, tag="psB",
                                 name=f"mx{t}")
                nc.tensor.matmul(mx_ps[:],
                                 emt8[:, t * 128:(t + 1) * 128],
                                 selm2[(t // 2) // 2][:],
                                 start=True, stop=True)
                xs = wpool.tile([128, 128], BF16, name=f"xs{t}")
                nc.vector.tensor_tensor(xs[:], xg[t][:], mx_ps[:],
                                        ALU.mult)
                xsel.append(xs)

            zm4 = [wpool.tile([128, 128], BF16, name=f"zm4{g}")
                   for g in range(4)]
            for g in range(4):
                zps = psB.tile([128, 128], F32, tag="psB", name=f"zps{g}")
                for k2xh in range(2):
                    t = g * 2 + k2xh
                    nc.tensor.matmul(zps[64 * k2xh:64 * k2xh + 64, :],
                                     w1fpbd[:], xsel[t][:],
                                     start=True, stop=True)
                for k2x in range(4):
                    nc.vector.tensor_tensor(
                        zm4[g][k2x * 32:(k2x + 1) * 32, :],
                        zps[k2x * 32:(k2x + 1) * 32, :],
                        tapv(m1p4, k2x * 32, 4 * g + k2x, ypitch),
                        ALU.mult)

            ym_ps = psC.tile([128, 64], F32, tag="psC", name="ym_ps")
            for g in range(4):
                nc.tensor.matmul(ym_ps[:], zm4[g][:],
                                 w2pk[:, g * 64:(g + 1) * 64],
                                 start=(g == 0), stop=(g == 3))
            ymm = wpool.tile([128, 64], BF16)
            nc.vector.tensor_tensor(ymm[:], ym_ps[:], m2g[:], ALU.mult)

            t2_ps = psB.tile([64, 128], BF16, tag="psB", name="t2_ps")
            nc.tensor.transpose(t2_ps[:], ymm[:], identb[:])
            ymmT = wpool.tile([64, 128], BF16)
            nc.scalar.copy(ymmT[:], t2_ps[:])

            # hopfield 2 in bf16
            a_ps = psA.tile([128, 512], F32, tag="psA", name="a_ps2")
            nc.tensor.matmul(a_ps[:], ymmT[:], ktb[:], start=True,
                             stop=True)
            rmax = wpool.tile([128, 1], F32, name="rmax2")
            nc.vector.tensor_reduce(rmax[:], a_ps[:], AX.X, ALU.max)
            negbm = wpool.tile([128, 1], F32, name="negbm2")
            nc.vector.tensor_scalar(negbm[:], rmax[:], -0.125, None,
                                    ALU.mult)
            p_sb = wpool.tile([128, 512], BF16, name="p_sb2")
            ssum = wpool.tile([128, 1], F32, name="ssum2")
            nc.scalar.activation(p_sb[:], a_ps[:], AF.Exp, bias=negbm[:],
                                 scale=0.125, accum_out=ssum[:])
            rec = wpool.tile([128, 1], F32, name="rec2")
            nc.vector.reciprocal(rec[:], ssum[:])
            nc.vector.tensor_scalar(p_sb[:], p_sb[:], rec[:], None,
                                    ALU.mult)
            yq2_ps = psC.tile([128, 64], F32, tag="psC", name="yq2_ps")
            for t in range(4):
                pt_ps = psB.tile([128, 128], BF16, tag="psB",
                                 name=f"pt2_{t}")
                nc.tensor.transpose(pt_ps[:],
                                    p_sb[:, t * 128:(t + 1) * 128],
                                    identb[:])
                pt_sb = wpool.tile([128, 128], BF16, tag="pt_sb2",
                                   name=f"pt_sb2{t}")
                nc.scalar.copy(pt_sb[:], pt_ps[:])
                nc.tensor.matmul(yq2_ps[:], pt_sb[:],
                                 vb[:, t * 64:(t + 1) * 64],
                                 start=(t == 0), stop=(t == 3))
            yq2 = wpool.tile([128, 64], F32)
            nc.scalar.copy(yq2[:], yq2_ps[:])

            tr_ps = psB.tile([64, 128], F32, tag="psB", name="tr_ps")
            nc.tensor.transpose(tr_ps[:], yq2[:], ident[:])
            outT = wpool.tile([64, 128], F32)
            nc.scalar.copy(outT[:], tr_ps[:])
            dmaS(AP(out_d, 0, [[64, 64], [4096, 2], [8, 8], [1, 8]]),
                 v(outT, 0, [[outT.ap[0][0], 64], [1, 128]]))

    return nc


_CACHE = {}


def kernel(**inputs) -> np.ndarray:
    from concourse.bass_utils import run_bass_kernel_spmd
    if "nc" not in _CACHE:
        from concourse import bacc
        nc = bacc.Bacc("TRN2", target_bir_lowering=False, debug=False,
                       num_devices=N_CORES)
        build_program(nc)
        nc.compile()
        _CACHE["nc"] = nc
    nc = _CACHE["nc"]
    feed = prepare_feed(inputs)
    in_maps = [dict(feed) for _ in range(N_CORES)]
    res = run_bass_kernel_spmd(nc, in_maps, list(range(N_CORES)))
    return np.asarray(res.results[0]["out"], np.float32)


# revision 13
# speedup vs baseline: 2.3534x; 1.2041x over previous
"""Trainium2 Bass kernel for nn_Block1_87144886436577 (vq_codebook), v2.

Analytic collapse of the reference's jacobians (see v1 docstring), with:
- conv1 as ONE fp32 matmul (im2col rows (ci,k1y,k1x) built by 3 DMAs
  from the host-padded image, replicated 4x in partition blocks),
- conv2 as 4 fp32 matmuls contracting (k2x, m)=128 per k2y over a
  shifted-row copy of y1,
- phase-B backprop (g1/ep) packed into 8+4 fp32 matmuls via
  host-rearranged weights; fp32 is REQUIRED upstream of the argmin
  (sel margins are ~1e-3; bf16 flips selections),
- the e-patch/sel DRAM round-trips deduplicated + spread across the
  sync/scalar/gpsimd DMA queues, argmin on a 64-partition layout, a
  permutation matmul so sel scatters with a single 3-dim DMA,
- everything downstream of sel (mask expansion, masked conv taps,
  second hopfield) in bf16 with tap-pair packing,
- all weight repacks/transposes done host-side (pure layout + casts).

All 8 cores run identical replicas; output read from core 0.
"""
import sys

import numpy as np

for _p in ("/opt/trn_rl_repo",):
    if _p not in sys.path:
        sys.path.insert(0, _p)

import concourse.bass as bass
import concourse.mybir as mybir
import concourse.tile as tile

F32 = mybir.dt.float32
BF16 = mybir.dt.bfloat16
U16 = mybir.dt.uint16
AF = mybir.ActivationFunctionType
ALU = mybir.AluOpType
AX = mybir.AxisListType
AP = bass.AP

N_CORES = 8


def v(t, off, pat):
    """Custom-view AP over a tile (t = AP returned by pool.tile)."""
    return AP(t.tensor, t.offset + off, pat)


def _e(r):
    return 1 if r >= 1 else 0


def _tables():
    """Input-independent index/mask tables."""
    import ml_dtypes
    BF = ml_dtypes.bfloat16

    ident = np.eye(128, dtype=np.float32)
    identb = ident.astype(BF)
    oidx128 = np.tile((np.arange(128) % 64).astype(BF)[None, :],
                      (128, 1))

    # xsel gather streams (same as v1): tile t=(k2y,k2xh); partition
    # p=k2xp*64+k1y*16+k1x*4+ci; j<128 per tile: (b,oy,ox).
    idxX = np.zeros((8, 128, 8), np.uint16)
    for t in range(8):
        k2y, k2xh = t // 2, t % 2
        for g in range(8):
            k2xp = g // 4
            k1y = g % 4
            k2x = 2 * k2xh + k2xp
            for j in range(128):
                b, oy, ox = j // 64, (j % 64) // 8, j % 8
                idxX[t, 16 * g + j % 16, j // 16] = (
                    b * 1444 + (4 * oy + 2 * k2y + k1y) * 38
                    + 4 * ox + 2 * k2x)
    ixX = np.zeros((128, 64), np.uint16)
    for t in range(8):
        ixX[:, t * 8:(t + 1) * 8] = idxX[t]

    # sel gather streams (as v1)
    idxS = np.zeros((2, 128, 8), np.uint16)
    for s in range(2):
        for g in range(8):
            k2yp, k2x = g // 4, g % 4
            k2y = s * 2 + k2yp
            for j in range(128):
                b, oy, ox = j // 64, (j % 64) // 8, j % 8
                idxS[s, 16 * g + j % 16, j // 16] = (
                    b * 1444 + (4 * oy + 2 * k2y) * 38 + 4 * ox + 2 * k2x)
    ixS = np.zeros((128, 16), np.uint16)
    for s in range(2):
        ixS[:, s * 8:(s + 1) * 8] = idxS[s]

    # mask expansion matrices, bf16: emt8[r, t*128+p]
    emt8 = np.zeros((128, 1024), np.float32)
    for t in range(8):
        k2y, k2xh = t // 2, t % 2
        for p in range(128):
            k2xp, k1y, k1x = p // 64, (p % 64) // 16, p % 4
            k2x = 2 * k2xh + k2xp
            r = (k2y % 2) * 64 + k2x * 16 + k1y * 4 + k1x
            emt8[r, t * 128 + p] = 1.0
    emt8 = emt8.astype(BF)

    # E9 gather (v1 layout): p = r*32+h*16+b*8+q (iy=4q+r, ix=16h+ixl),
    # stream j = ixl*9 + jj. data row = ed4p[b, q+e(r) : +3 rows] flat.
    idxE = np.zeros((128, 9), np.uint16)
    oidx9 = np.full((128, 144), 3000.0, np.float32)
    zc128 = np.zeros((128, 16), np.float32)
    for r in range(4):
        for h in range(2):
            g = r * 2 + h
            for j in range(144):
                ixl, jj = j // 9, j % 9
                jy, jx = jj // 3, jj % 3
                t_ = ixl % 4
                s = 4 * h + ixl // 4
                dy = r - 4 * _e(r) + 4 * jy + 3
                dx = t_ - 4 * _e(t_) + 4 * jx + 3
                ox = s + _e(t_) - jx
                if 0 <= dy < 10 and 0 <= dx < 10:
                    idx = jy * 120 + (ox + 2) * 10 + dx
                else:
                    idx = 0  # guaranteed-zero pad cell (oxp=0 col)
                idxE[16 * g + j % 16, j // 16] = idx
    for r in range(4):
        for h in range(2):
            for b in range(2):
                for q in range(8):
                    p = r * 32 + h * 16 + b * 8 + q
                    iy = 4 * q + r
                    for ixl in range(16):
                        ix = 16 * h + ixl
                        t_ = ix % 4
                        s = ix // 4
                        for jj in range(9):
                            jy, jx = jj // 3, jj % 3
                            oy = q + _e(r) - jy
                            ox = s + _e(t_) - jx
                            dy = iy - 4 * oy + 3
                            dx = ix - 4 * ox + 3
                            if (0 <= oy < 8 and 0 <= ox < 8
                                    and 0 <= dy < 10 and 0 <= dx < 10):
                                oidx9[p, ixl * 9 + jj] = oy * 8 + ox
                        for o in range(64):
                            oy, ox = o // 8, o % 8
                            if not (0 <= iy - 4 * oy + 3 < 10
                                    and 0 <= ix - 4 * ox + 3 < 10):
                                zc128[p, ixl] = float(o)
                                break

    # permutations: sel16 rows (r,h,b,q) -> rows (b, iy), cols ix-halves
    permA = np.zeros((128, 64), np.float32)
    permB = np.zeros((128, 64), np.float32)
    for r in range(4):
        for h in range(2):
            for b in range(2):
                for q in range(8):
                    p = r * 32 + h * 16 + b * 8 + q
                    (permA if h == 0 else permB)[p, b * 32 + 4 * q + r] \
                        = 1.0

    neg1s = np.full((2, 1520), -1.0, BF)
    zed = np.zeros((12, 2400), np.float32)
    return {"ident": ident, "identb": identb, "oidx128": oidx128,
            "ixX": ixX, "ixS": ixS, "emt8": emt8, "idxE": idxE,
            "oidx9": oidx9, "zc128": zc128, "permA": permA,
            "permB": permB, "neg1s": neg1s, "zed": zed}


def prepare_feed(inputs):
    """Host-side layout/cast-only rearrangements of the inputs."""
    import ml_dtypes
    BF = ml_dtypes.bfloat16
    x = np.asarray(inputs["x"], np.float32)    # (2,3,32,32)
    w1 = np.asarray(inputs["w1"], np.float32)  # (32,3,4,4)
    b1 = np.asarray(inputs["b1"], np.float32)  # (32,)
    w2 = np.asarray(inputs["w2"], np.float32)  # (64,32,4,4)
    b2 = np.asarray(inputs["b2"], np.float32)  # (64,)
    K = np.asarray(inputs["K"], np.float32)    # (512,64)
    V = np.asarray(inputs["V"], np.float32)    # (512,64)

    f = dict(_TABLES)

    xpad = np.zeros((3, 3008), np.float32)
    img = np.zeros((3, 2, 38, 38), np.float32)
    img[:, :, 3:35, 3:35] = x.transpose(1, 0, 2, 3)
    xpad[:, 0:2888] = img.reshape(3, 2888)
    f["xpad3"] = xpad

    # data_x rows (ci,k1x): padded image shifted left by k1x; ci=3 zero.
    dx16 = np.zeros((16, 2888), np.float32)
    for ci in range(3):
        for k1x in range(4):
            dx16[ci * 4 + k1x, 0:2888 - k1x] = \
                f["xpad3"][ci, k1x:2888]
    f["dx16"] = dx16.astype(BF)

    # conv1 weights: [ci*16+k1y*4+k1x, j*32+m], 4 dup col blocks
    w1t = w1.transpose(1, 2, 3, 0).reshape(48, 32)
    f["w1f4"] = np.tile(w1t, (1, 4)).copy()
    f["b1t4"] = np.tile(b1, 4).reshape(128, 1).copy()

    # conv2 weights: [k2x*32+m, k2y*64+c]
    W2p = np.zeros((128, 256), np.float32)
    for k2y in range(4):
        for k2x in range(4):
            W2p[k2x * 32:(k2x + 1) * 32, k2y * 64:(k2y + 1) * 64] = \
                w2[:, :, k2y, k2x].T
    f["W2p"] = W2p
    f["b2t"] = b2.reshape(64, 1).copy()

    f["ktK"] = K.T.copy()                       # [64, 512]
    f["vV"] = V.reshape(4, 128, 64).transpose(1, 0, 2).reshape(128, 256) \
        .copy()                                 # [128, (t,c)]
    f["ktb"] = f["ktK"].astype(BF)
    f["vb"] = f["vV"].astype(BF)

    # g1 pair weights: [c, k2*32+m]
    w2k2m = np.zeros((64, 512), np.float32)
    for k2 in range(16):
        k2y, k2x = k2 // 4, k2 % 4
        w2k2m[:, k2 * 32:(k2 + 1) * 32] = w2[:, :, k2y, k2x]
    f["w2k2m"] = w2k2m

    # w1 summed-tap images, built on chip from w1sb4
    f["w1sb4"] = np.tile(w1.reshape(32, 48), (4, 1)).copy()

    # z-pair blockdiag weights bf16 [128, 64]
    w1fp64 = np.zeros((64, 32), np.float32)
    for k1y in range(4):
        for ci in range(3):
            for k1x in range(4):
                w1fp64[k1y * 16 + ci * 4 + k1x, :] = w1[:, ci, k1y, k1x]
    bd = np.zeros((128, 64), np.float32)
    bd[0:64, 0:32] = w1fp64
    bd[64:128, 32:64] = w1fp64
    f["w1fpbd"] = bd.astype(BF)

    # ym pack weights bf16 [128, (g,c)]
    w2pk = np.zeros((128, 256), np.float32)
    for g in range(4):
        for k2x in range(4):
            w2pk[k2x * 32:(k2x + 1) * 32, g * 64:(g + 1) * 64] = \
                w2[:, :, g, k2x].T
    f["w2pk"] = w2pk.astype(BF)
    return f


_TABLES = _tables()


def build_program(nc):
    def P(name, shape, dt):
        return nc.declare_dram_parameter(name, shape, dt, isOutput=False)

    xpad_d = P("xpad3", [3, 3008], F32)
    dx16_d = P("dx16", [16, 2888], BF16)
    w1f4_d = P("w1f4", [48, 128], F32)
    b1t4_d = P("b1t4", [128, 1], F32)
    W2p_d = P("W2p", [128, 256], F32)
    b2t_d = P("b2t", [64, 1], F32)
    ktK_d = P("ktK", [64, 512], F32)
    vV_d = P("vV", [128, 256], F32)
    ktb_d = P("ktb", [64, 512], BF16)
    vb_d = P("vb", [128, 256], BF16)
    w2k2m_d = P("w2k2m", [64, 512], F32)
    w1sb4_d = P("w1sb4", [128, 48], F32)
    w1fpbd_d = P("w1fpbd", [128, 64], BF16)
    w2pk_d = P("w2pk", [128, 256], BF16)
    ident_d = P("ident", [128, 128], F32)
    identb_d = P("identb", [128, 128], BF16)
    oidx128_d = P("oidx128", [128, 128], BF16)
    ixX_d = P("ixX", [128, 64], U16)
    ixS_d = P("ixS", [128, 16], U16)
    emt8_d = P("emt8", [128, 1024], BF16)
    idxE_d = P("idxE", [128, 9], U16)
    oidx9_d = P("oidx9", [128, 144], F32)
    zc128_d = P("zc128", [128, 16], F32)
    permA_d = P("permA", [128, 64], F32)
    permB_d = P("permB", [128, 64], F32)
    neg1s_d = P("neg1s", [2, 1520], BF16)
    zed_d = P("zed", [12, 2400], F32)
    out_d = nc.declare_dram_parameter("out", [2, 64, 8, 8], F32,
                                      isOutput=True)

    with tile.TileContext(nc) as tc:
        with (
            tc.tile_pool(name="const", bufs=1) as cpool,
            tc.tile_pool(name="work", bufs=1) as wpool,
            tc.tile_pool(name="psA", bufs=2, space="PSUM") as psA,
            tc.tile_pool(name="psB", bufs=4, space="PSUM") as psB,
            tc.tile_pool(name="psC", bufs=2, space="PSUM") as psC,
            tc.tile_pool(name="dram", bufs=1, space="DRAM") as dpool,
        ):
            dmaS = nc.sync.dma_start
            dmaA = nc.scalar.dma_start
            dmaG = nc.gpsimd.dma_start

            # ---- DRAM scratch ----
            ed4p = dpool.tile([2, 12, 12, 100], F32)
            edT = dpool.tile([24, 1200], F32)
            sel_pad = dpool.tile([3040], BF16)

            # ---- critical-path loads first (sync queue) ----
            w1f4 = wpool.tile([48, 128], F32)
            dmaS(w1f4[:], w1f4_d[:])
            X48 = wpool.tile([48, 2888], F32)
            for ci, dq in ((0, dmaS), (1, dmaA), (2, dmaS)):
                dq(X48[ci * 16:(ci + 1) * 16, :],
                   AP(xpad_d, ci * 3008, [[38, 4], [1, 4], [1, 2888]]))
            b1t4 = wpool.tile([128, 1], F32)
            dmaS(b1t4[:], b1t4_d[:])
            W2p = wpool.tile([128, 256], F32)
            dmaS(W2p[:], W2p_d[:])
            b2t = wpool.tile([64, 1], F32)
            dmaS(b2t[:], b2t_d[:])
            ktK = wpool.tile([64, 512], F32)
            dmaS(ktK[:], ktK_d[:])
            ident = cpool.tile([128, 128], F32)
            dmaS(ident[:], ident_d[:])
            vV = wpool.tile([128, 256], F32)
            dmaS(vV[:], vV_d[:])
            w2k2m = wpool.tile([64, 512], F32)
            dmaA(w2k2m[:], w2k2m_d[:])
            w1sb4 = wpool.tile([128, 48], F32)
            dmaA(w1sb4[:], w1sb4_d[:])

            # ---- non-critical loads (scalar/gpsimd queues) ----
            dmaA(v(ed4p, 0, [[2400, 12], [1, 2400]]), zed_d[:])
            dmaA(v(sel_pad, 0, [[1520, 2], [1, 1520]]), neg1s_d[:])
            ixX = cpool.tile([128, 64], U16)
            dmaS(ixX[:], ixX_d[:])
            dx = wpool.tile([128, 2888], BF16)
            dmaS(dx[0:16, :], dx16_d[:])
            for d in (16, 32, 64):
                dmaS(dx[d:2 * d, :], dx[0:d, :])
            identb = cpool.tile([128, 128], BF16)
            dmaA(identb[:], identb_d[:])
            oidx128 = cpool.tile([128, 128], BF16)
            dmaA(oidx128[:], oidx128_d[:])
            ixS = cpool.tile([128, 16], U16)
            dmaA(ixS[:], ixS_d[:])
            emt8 = cpool.tile([128, 1024], BF16)
            dmaA(emt8[:], emt8_d[:])
            idxE = cpool.tile([128, 9], U16)
            dmaA(idxE[:], idxE_d[:])
            oidx9 = cpool.tile([128, 144], F32)
            dmaA(oidx9[:], oidx9_d[:])
            zc128 = cpool.tile([128, 16], F32)
            dmaA(zc128[:], zc128_d[:])
            permA = cpool.tile([128, 64], F32)
            dmaA(permA[:], permA_d[:])
            permB = cpool.tile([128, 64], F32)
            dmaA(permB[:], permB_d[:])
            ktb = wpool.tile([64, 512], BF16)
            dmaA(ktb[:], ktb_d[:])
            vb = wpool.tile([128, 256], BF16)
            dmaA(vb[:], vb_d[:])
            w1fpbd = wpool.tile([128, 64], BF16)
            dmaA(w1fpbd[:], w1fpbd_d[:])
            w2pk = wpool.tile([128, 256], BF16)
            dmaA(w2pk[:], w2pk_d[:])

            # x-side gather (one batched indirect, overlaps A/B)
            xga = wpool.tile([128, 1024], BF16)
            nc.gpsimd.indirect_copy(
                v(xga, 0, [[xga.ap[0][0], 128], [1, 1024], [1, 1]]),
                dx[:], ixX[:, 0:64], True)

            # w1 summed-tap images -> w1stp4[g] [128(k2x,m), 100]
            w1s4 = wpool.tile([128, 16], F32)
            nc.vector.tensor_reduce(
                w1s4[:],
                v(w1sb4, 0, [[w1sb4.ap[0][0], 128], [1, 16], [16, 3]]),
                AX.X, ALU.add)
            w1si = wpool.tile([128, 256], F32)
            nc.vector.memset(w1si[:], 0.0)
            nc.vector.tensor_copy(
                v(w1si, 102, [[w1si.ap[0][0], 128], [16, 4], [1, 4]]),
                w1s4[:])
            w1stp4 = [wpool.tile([128, 100], F32, name=f"w1stp4{g}")
                      for g in range(4)]
            wpitch = w1si.ap[0][0]
            for g in range(4):
                for k2x in range(4):
                    dst = w1stp4[g]
                    nc.vector.tensor_copy(
                        v(dst, k2x * 32 * dst.ap[0][0],
                          [[dst.ap[0][0], 32], [10, 10], [1, 10]]),
                        v(w1si, k2x * 32 * wpitch
                          + (6 - 2 * g) * 16 + 6 - 2 * k2x,
                          [[wpitch, 32], [16, 10], [1, 10]]))

            # ---- Phase A: conv1 (one matmul) ----
            y1ps = psA.tile([128, 512], F32, tag="psA", name="y1ps")
            nc.tensor.matmul(
                y1ps[:], w1f4[:],
                v(X48, 78, [[X48.ap[0][0], 48], [1444, 2], [76, 16],
                            [2, 16]]),
                start=True, stop=True)
            y1sb = wpool.tile([128, 512], F32)
            nc.scalar.activation(y1sb[:], y1ps[:], AF.Relu, bias=b1t4[:])

            y1p4 = wpool.tile([128, 648], F32)
            nc.vector.memset(y1p4[:], 0.0)
            ypitch = y1p4.ap[0][0]
            nc.vector.tensor_copy(
                v(y1p4, 19, [[ypitch, 128], [324, 2], [18, 16], [1, 16]]),
                v(y1sb, 0,
                  [[y1sb.ap[0][0], 128], [256, 2], [16, 16], [1, 16]]))
            m1p4 = wpool.tile([128, 648], F32)
            nc.vector.tensor_scalar(m1p4[:], y1p4[:], 0.0, None, ALU.is_gt)

            # shifted-row copy for conv2 im2col (block k2x shifted by k2x)
            Y2 = wpool.tile([128, 648], F32)
            nc.vector.tensor_copy(Y2[0:32, :], y1p4[0:32, :])
            nc.vector.tensor_copy(Y2[32:64, 0:647], y1p4[32:64, 1:648])
            nc.scalar.copy(Y2[64:96, 0:646], y1p4[64:96, 2:648])
            nc.scalar.copy(Y2[96:128, 0:645], y1p4[96:128, 3:648])

            def tapv(tl, pbase, k2, pitch, n=32):
                k2y, k2x = k2 // 4, k2 % 4
                return v(tl, pbase * pitch + 18 * k2y + k2x,
                         [[pitch, n], [324, 2], [36, 8], [2, 8]])

            # ---- conv2: 4 matmuls ----
            ypre = psA.tile([64, 128], F32, tag="psA", name="ypre")
            for k2y in range(4):
                nc.tensor.matmul(
                    ypre[:],
                    v(W2p, k2y * 64, [[W2p.ap[0][0], 128], [1, 64]]),
                    v(Y2, 18 * k2y,
                      [[Y2.ap[0][0], 128], [324, 2], [36, 8], [2, 8]]),
                    start=(k2y == 0), stop=(k2y == 3))
            yT = wpool.tile([64, 128], F32)
            nc.scalar.activation(yT[:], ypre[:], AF.Relu, bias=b2t[:])
            m2T = wpool.tile([64, 128], F32)
            nc.vector.tensor_scalar(m2T[:], yT[:], 0.0, None, ALU.is_gt)

            # m2 mask in (b,o)-partition layout for phase C
            yTT_ps = psB.tile([128, 64], F32, tag="psB", name="yTT_ps")
            nc.tensor.transpose(yTT_ps[:], yT[:], ident[0:64, 0:64])
            m2g = wpool.tile([128, 64], F32)
            nc.vector.tensor_scalar(m2g[:], yTT_ps[:], 0.0, None, ALU.is_gt)

            # packed per-group m1 masks (built on idle DVE during hop1)
            m1pk = [wpool.tile([128, 128], F32, name=f"m1pk{g}")
                    for g in range(4)]
            for g in range(4):
                for k2x in range(4):
                    nc.vector.tensor_copy(
                        m1pk[g][k2x * 32:(k2x + 1) * 32, :],
                        tapv(m1p4, k2x * 32, 4 * g + k2x, ypitch))

            def hopfield_f32(src, tag):
                a_ps = psA.tile([128, 512], F32, tag="psA",
                                name=f"a_ps{tag}")
                nc.tensor.matmul(a_ps[:], src, ktK[:], start=True,
                                 stop=True)
                rmax = wpool.tile([128, 1], F32, name=f"rmax{tag}")
                nc.vector.tensor_reduce(rmax[:], a_ps[:], AX.X, ALU.max)
                negbm = wpool.tile([128, 1], F32, name=f"negbm{tag}")
                nc.vector.tensor_scalar(negbm[:], rmax[:], -0.125, None,
                                        ALU.mult)
                p_sb = wpool.tile([128, 512], F32, name=f"p_sb{tag}")
                ssum = wpool.tile([128, 1], F32, name=f"ssum{tag}")
                nc.scalar.activation(p_sb[:], a_ps[:], AF.Exp,
                                     bias=negbm[:], scale=0.125,
                                     accum_out=ssum[:])
                rec = wpool.tile([128, 1], F32, name=f"rec{tag}")
                nc.vector.reciprocal(rec[:], ssum[:])
                nc.vector.tensor_scalar(p_sb[:], p_sb[:], rec[:], None,
                                        ALU.mult)
                yq_ps = psC.tile([128, 64], F32, tag="psC",
                                 name=f"yq_ps{tag}")
                for t in range(4):
                    pt_ps = psB.tile([128, 128], F32, tag="psB",
                                     name=f"pt_ps{tag}{t}")
                    nc.tensor.transpose(pt_ps[:],
                                        p_sb[:, t * 128:(t + 1) * 128],
                                        ident[:])
                    pt_sb = wpool.tile([128, 128], F32, tag="pt_sb",
                                       name=f"pt_sb{tag}{t}")
                    nc.scalar.copy(pt_sb[:], pt_ps[:])
                    nc.tensor.matmul(yq_ps[:], pt_sb[:],
                                     vV[:, t * 64:(t + 1) * 64],
                                     start=(t == 0), stop=(t == 3))
                yq_sb = wpool.tile([128, 64], F32, name=f"yq_sb{tag}")
                nc.scalar.copy(yq_sb[:], yq_ps[:])
                return yq_sb

            yq1 = hopfield_f32(yT[:], "h1")

            yqT_ps = psB.tile([64, 128], F32, tag="psB", name="yqT_ps")
            nc.tensor.transpose(yqT_ps[:], yq1[:], ident[:])
            r2T = wpool.tile([64, 128], F32)
            nc.vector.scalar_tensor_tensor(r2T[:], yqT_ps[:], -1.0, yT[:],
                                           ALU.mult, ALU.add)
            nc.vector.tensor_mul(r2T[:], r2T[:], m2T[:])

            # ---- Phase B: g1 pairs + ep ----
            g1m4 = [wpool.tile([128, 128], F32, name=f"g1m4{g}")
                    for g in range(4)]
            for g in range(4):
                g1ps = psB.tile([128, 128], F32, tag="psB",
                                name=f"g1ps{g}")
                for j in range(2):
                    nc.tensor.matmul(
                        g1ps[64 * j:64 * j + 64, :],
                        v(w2k2m, (2 * g + j) * 64,
                          [[w2k2m.ap[0][0], 64], [1, 64]]),
                        r2T[:], start=True, stop=True)
                nc.vector.tensor_tensor(g1m4[g][:], g1ps[:],
                                        m1pk[g][:], ALU.mult)

            ep_ps = psA.tile([100, 128], F32, tag="psA", name="ep_ps")
            for g in range(4):
                nc.tensor.matmul(ep_ps[:], w1stp4[g][:], g1m4[g][:],
                                 start=(g == 0), stop=(g == 3))
            ep_sb = wpool.tile([100, 128], F32)
            nc.scalar.copy(ep_sb[:], ep_ps[:])
            ep2_ps = psB.tile([128, 100], F32, tag="psB", name="ep2_ps")
            nc.tensor.transpose(ep2_ps[:], ep_sb[:], ident[0:100, 0:100])
            ep2 = wpool.tile([128, 100], F32)
            nc.scalar.copy(ep2[:], ep2_ps[:])

            # scatter e-patches to DRAM (single 4-dim DMA)
            for b, dq in ((0, dmaS), (1, dmaA)):
                dq(v(ed4p, b * 14400 + 2 * 1200 + 2 * 100,
                     [[1200, 8], [100, 8], [1, 100]]),
                   ep2[b * 64:(b + 1) * 64, :])

            # data_e (dedup: two distinct 16-row contents + 2 copies)
            # compact e-patch windows: one clean image load, DVE
            # reorder (oxp between dy and dx), write back, 16 tiny loads
            edS = wpool.tile([24, 1200], F32)
            dmaS(edS[:], v(ed4p, 0, [[1200, 24], [1, 1200]]))
            edR = wpool.tile([24, 1200], F32)
            nc.vector.tensor_copy(
                v(edR, 0, [[edR.ap[0][0], 24], [1, 1200]]),
                v(edS, 0, [[edS.ap[0][0], 24], [10, 10], [100, 12],
                           [1, 10]]))
            dmaS(v(edT, 0, [[1200, 24], [1, 1200]]),
                 edR[0:24, :])
            de = wpool.tile([128, 360], F32)
            for g in range(8):
                r, h = g // 2, g % 2
                e = 1 if r >= 1 else 0
                for b in range(2):
                    dq = dmaS if (g + b) % 2 == 0 else dmaA
                    dq(de[g * 16 + b * 8:g * 16 + b * 8 + 8, :],
                       v(edT, (b * 12 + e + 2) * 1200
                         + (r - 4 * e + 3) * 120,
                         [[1200, 8], [-720, 3], [1, 120]]))

            e9 = wpool.tile([128, 144], F32)
            e9p = e9.ap[0][0]
            nc.gpsimd.indirect_copy(
                v(e9, 0, [[e9p, 128], [1, 144], [1, 1]]),
                de[:], idxE[:], True)

            # argmin with reference tie semantics (v1 128-row layout)
            mincand = wpool.tile([128, 16], F32)
            nc.vector.tensor_reduce(
                mincand[:], v(e9, 0, [[e9p, 128], [9, 16], [1, 9]]),
                AX.X, ALU.min)
            mstar = wpool.tile([128, 16], F32)
            nc.vector.tensor_scalar(mstar[:], mincand[:], 0.0, None,
                                    ALU.min)
            eq9 = wpool.tile([128, 144], F32)
            nc.vector.tensor_tensor(
                v(eq9, 0, [[eq9.ap[0][0], 128], [9, 16], [1, 9]]),
                v(e9, 0, [[e9p, 128], [9, 16], [1, 9]]),
                v(mstar, 0, [[mstar.ap[0][0], 128], [1, 16], [0, 9]]),
                ALU.is_equal)
            cs = wpool.tile([128, 144], F32)
            nc.vector.scalar_tensor_tensor(cs[:], eq9[:], -1000.0,
                                           oidx9[:], ALU.mult, ALU.add)
            minc2 = wpool.tile([128, 16], F32)
            nc.vector.tensor_reduce(
                minc2[:], v(cs, 0, [[cs.ap[0][0], 128], [9, 16], [1, 9]]),
                AX.X, ALU.min)
            zeq = wpool.tile([128, 16], F32)
            nc.vector.tensor_scalar(zeq[:], mstar[:], 0.0, None,
                                    ALU.is_equal)
            zsc = wpool.tile([128, 16], F32)
            nc.vector.scalar_tensor_tensor(zsc[:], zeq[:], -1000.0,
                                           zc128[:], ALU.mult, ALU.add)
            sel16 = wpool.tile([128, 16], F32)
            nc.vector.tensor_tensor(sel16[:], minc2[:], zsc[:], ALU.min)
            nc.vector.tensor_scalar(sel16[:], sel16[:], 1000.0, None,
                                    ALU.add)

            # permute rows (r,h,b,q)->(b,iy) x ix-halves, ONE 3-dim scatter
            selbi_ps = psC.tile([64, 32], F32, tag="psC", name="selbi_ps")
            nc.tensor.matmul(selbi_ps[:, 0:16], permA[:], sel16[:],
                             start=True, stop=True)
            nc.tensor.matmul(selbi_ps[:, 16:32], permB[:], sel16[:],
                             start=True, stop=True)
            selbi = wpool.tile([64, 32], BF16)
            nc.scalar.copy(selbi[:], selbi_ps[:])
            dmaS(v(sel_pad, 117, [[1444, 2], [38, 32], [1, 32]]),
                 v(selbi, 0, [[selbi.ap[0][0], 64], [1, 32]]))

            # sel image rows with (k1y,k1x) shifts, replicated x8
            ds = wpool.tile([128, 2888], BF16)
            qs = (dmaS, dmaA, dmaS, dmaA, dmaS, dmaA, dmaS, dmaA)
            for k in range(8):
                qs[k](ds[k * 16:(k + 1) * 16, :],
                      v(sel_pad, 0, [[38, 4], [1, 4], [1, 2888]]))

            sg = wpool.tile([128, 256], BF16)
            nc.gpsimd.indirect_copy(
                v(sg, 0, [[sg.ap[0][0], 128], [1, 256], [1, 1]]),
                ds[:], ixS[:, 0:16], True)
            sgb = wpool.tile([128, 256], BF16)
            nc.vector.tensor_tensor(
                sgb[:], sg[:],
                v(oidx128, 0, [[oidx128.ap[0][0], 128], [0, 2], [1, 128]]),
                ALU.is_equal)
            selm2 = [sgb[:, 0:128], sgb[:, 128:256]]

            # ---- Phase C (bf16): masks -> xsel -> z pairs -> ym ----
            xsel = []
            for t in range(8):
                mx_ps = psB.tile([128, 128], F32, tag="psB",
                                 name=f"mx{t}")
                nc.tensor.matmul(mx_ps[:],
                                 emt8[:, t * 128:(t + 1) * 128],
                                 selm2[(t // 2) // 2],
                                 start=True, stop=True)
                xs = wpool.tile([128, 128], BF16, name=f"xs{t}")
                nc.vector.tensor_tensor(xs[:], xga[:, t * 128:(t + 1) * 128],
                                        mx_ps[:], ALU.mult)
                xsel.append(xs)

            zm4 = [wpool.tile([128, 128], BF16, name=f"zm4{g}")
                   for g in range(4)]
            for g in range(4):
                zps = psB.tile([128, 128], F32, tag="psB", name=f"zps{g}")
                for k2xh in range(2):
                    t = g * 2 + k2xh
                    nc.tensor.matmul(zps[64 * k2xh:64 * k2xh + 64, :],
                                     w1fpbd[:], xsel[t][:],
                                     start=True, stop=True)
                nc.vector.tensor_tensor(zm4[g][:], zps[:],
                                        m1pk[g][:], ALU.mult)

            ym_ps = psC.tile([128, 64], F32, tag="psC", name="ym_ps")
            for g in range(4):
                nc.tensor.matmul(ym_ps[:], zm4[g][:],
                                 w2pk[:, g * 64:(g + 1) * 64],
                                 start=(g == 0), stop=(g == 3))
            ymm = wpool.tile([128, 64], BF16)
            nc.vector.tensor_tensor(ymm[:], ym_ps[:], m2g[:], ALU.mult)

            t2_ps = psB.tile([64, 128], BF16, tag="psB", name="t2_ps")
            nc.tensor.transpose(t2_ps[:], ymm[:], identb[:])
            ymmT = wpool.tile([64, 128], BF16)
            nc.scalar.copy(ymmT[:], t2_ps[:])

            # hopfield 2 in bf16
            a_ps = psA.tile([128, 512], F32, tag="psA", name="a_ps2")
            nc.tensor.matmul(a_ps[:], ymmT[:], ktb[:], start=True,
                             stop=True)
            rmax = wpool.tile([128, 1], F32, name="rmax2")
            nc.vector.tensor_reduce(rmax[:], a_ps[:], AX.X, ALU.max)
            negbm = wpool.tile([128, 1], F32, name="negbm2")
            nc.vector.tensor_scalar(negbm[:], rmax[:], -0.125, None,
                                    ALU.mult)
            p_sb = wpool.tile([128, 512], BF16, name="p_sb2")
            ssum = wpool.tile([128, 1], F32, name="ssum2")
            nc.scalar.activation(p_sb[:], a_ps[:], AF.Exp, bias=negbm[:],
                                 scale=0.125, accum_out=ssum[:])
            rec = wpool.tile([128, 1], F32, name="rec2")
            nc.vector.reciprocal(rec[:], ssum[:])
            nc.vector.tensor_scalar(p_sb[:], p_sb[:], rec[:], None,
                                    ALU.mult)
            yq2_ps = psC.tile([128, 64], F32, tag="psC", name="yq2_ps")
            for t in range(4):
                pt_ps = psB.tile([128, 128], BF16, tag="psB",
                                 name=f"pt2_{t}")
                nc.tensor.transpose(pt_ps[:],
                                    p_sb[:, t * 128:(t + 1) * 128],
                                    identb[:])
                pt_sb = wpool.tile([128, 128], BF16, tag="pt_sb2",
                                   name=f"pt_sb2{t}")
                nc.scalar.copy(pt_sb[:], pt_ps[:])
                nc.tensor.matmul(yq2_ps[:], pt_sb[:],
                                 vb[:, t * 64:(t + 1) * 64],
                                 start=(t == 0), stop=(t == 3))
            yq2 = wpool.tile([128, 64], F32)
            nc.scalar.copy(yq2[:], yq2_ps[:])

            tr_ps = psB.tile([64, 128], F32, tag="psB", name="tr_ps")
            nc.tensor.transpose(tr_ps[:], yq2[:], ident[:])
            outT = wpool.tile([64, 128], F32)
            nc.scalar.copy(outT[:], tr_ps[:])
            dmaS(AP(out_d, 0, [[64, 64], [4096, 2], [8, 8], [1, 8]]),
                 v(outT, 0, [[outT.ap[0][0], 64], [1, 128]]))

    return nc


_CACHE = {}


def kernel(**inputs) -> np.ndarray:
    from concourse.bass_utils import run_bass_kernel_spmd
    if "nc" not in _CACHE:
        from concourse import bacc
        nc = bacc.Bacc("TRN2", target_bir_lowering=False, debug=False,
                       num_devices=N_CORES)
        build_program(nc)
        nc.compile()
        _CACHE["nc"] = nc
    nc = _CACHE["nc"]
    feed = prepare_feed(inputs)
    in_maps = [dict(feed) for _ in range(N_CORES)]
    res = run_bass_kernel_spmd(nc, in_maps, list(range(N_CORES)))
    return np.asarray(res.results[0]["out"], np.float32)
